# revision 18
# baseline (speedup 1.0000x reference)
import sys

for _p in ("/opt/trn_rl_repo",):
    if _p not in sys.path:
        sys.path.insert(0, _p)

import numpy as np
import ml_dtypes
import bass_rust
import concourse.bass as bass
import concourse.mybir as mybir
import concourse.tile as tile
from concourse.bass_utils import run_bass_kernel_spmd

DT = mybir.dt
F8 = ml_dtypes.float8_e4m3
DR = mybir.MatmulPerfMode.DoubleRow

# Problem constants (hardcoded from the nn_AutoFlow spec)
B, D, NH0, NH1, L = 32768, 64, 256, 256, 16
NCORES = 8
BC = B // NCORES          # 4096 samples per core
BT = 512                  # batch tile (free dim of activation tiles)
NT = BC // BT             # tiles per core
NP = NT // 2              # tile pairs: tile 2p -> partitions 0:64, 2p+1 -> 64:128
WARM_MMS = 40
WARM_N = 128

# fp8 weight blob column layout, per layer (DoubleRow pairs flattened as
# (k m) so rearrange("p (k m) -> p k m", k=2) recovers the pair)
# mm0 net n chunk m: [64p, 2x128] = (A0 m-chunk | b0 row) at n*512 + m*256
#   (rows 64:128 hold a copy for odd tiles whose y sits at partitions 64:128)
# mm1 net n chunk m: [128p, 2x128] = (A1 k0 mcols | A1 k1) at 1024+n*512+m*256
# mm2 net n:         [128p, 2x64]  = (A2 k0 | A2 k1)       at 2048 + n*128
# b1 prebias (n,m):   [1p, 2x128]  = (b1 chunk | zeros)     at 2304 + (n*2+m)*256
CW8 = 3328

# fp16 blob: per layer col 0:64 = -b2l row (partitions 0 and 64)
CW16 = 64

# const fp16 blob [128, 576]: cols 0:64 = I64 (rows 0:64 and 64:128),
# cols 64:576 = 1.0
CONST_COLS = 576

# fp32 bias blob, per layer 5 cols:
# 0: b1_loc[0:128]  1: b1_loc[128:256]  2: b1_sc[0:128]  3: b1_sc[128:256]
# 4: -b2s (rows 0:64 and 64:128)
CB = 5

# engines for the per-(layer,tile) psum-draining ops. Only ACT ("act") and
# DVE ("dve") may touch PSUM; gpsimd/Pool is SBUF-only on this stack.
RELU0_ENG = {0: "act", 1: "split"}
RELU1_ENG = {0: "act", 1: "dve"}
RELU_SPLIT_C = 96    # "split": flattened cols [0:C] on ACT, [C:2*BT] on DVE
HBT = BT // 2        # coupling psum chunk width


def _patch_tile_drain(maxw=1):
    """walrus on this stack allows only 1 sync-wait on the kernel-tail Drain;
    split the TileContext drain's waits across a chain of drains."""
    from concourse.tile import ScopedClock

    def _drain_and_barrier(self, tick_clock, wait_clock):
        drain_inst = self.nc.sync.drain()
        wait_clock.add_sem_waits(
            drain_inst.ins, ScopedClock({None: tick_clock.global_clock})
        )
        inst = drain_inst.ins
        si = inst.sync_info
        if si is not None:
            waits = list(si.on_wait)
            ups = list(si.on_update)
            if len(waits) > maxw:
                chunks = [waits[i:i + maxw] for i in range(0, len(waits), maxw)]
                inst.sync_info = bass_rust.SyncInfo(on_wait=chunks[0], on_update=[])
                for j, chunk in enumerate(chunks[1:]):
                    extra = self.nc.sync.drain().ins
                    is_last = j == len(chunks) - 2
                    extra.sync_info = bass_rust.SyncInfo(
                        on_wait=chunk, on_update=ups if is_last else []
                    )
        self.nc.all_engine_barrier()
        assert self.sems is not None
        popped = self.nc._tile_sem_poison_stack.pop()
        assert popped is self._sem_poison
        self.nc.clear_and_free_semaphores(list(self.sems.allocated().values()))
        self.nc.all_engine_barrier()

    tile.TileContext._drain_and_barrier = _drain_and_barrier


_MAXW1_TYPES = ("InstDrain", "InstActivation")


def _split_excess_waits(nc, maxw=1):
    """walrus on this stack encodes very few semaphore-wait slots per
    instruction. Spill excess waits onto same-engine NoOps inserted just
    before the instruction (engine streams are in-order, so this is
    equivalent)."""
    for f in nc.m.functions:
        for bb in f.blocks:
            il = bb.instructions
            out = []
            for inst in il:
                si = getattr(inst, "sync_info", None)
                mw = 1 if type(inst).__name__ in _MAXW1_TYPES else maxw
                if si is not None and len(si.on_wait) > mw:
                    waits = list(si.on_wait)
                    ups = list(si.on_update)
                    chunks = [waits[i:i + mw] for i in range(0, len(waits), mw)]
                    for j, ch in enumerate(chunks[:-1]):
                        nop = mybir.InstNoOp(
                            name=f"{inst.name}-wsp{j}", ins=[], outs=[]
                        )
                        nop.engine = inst.engine
                        nop.sync_info = bass_rust.SyncInfo(on_wait=ch, on_update=[])
                        nc.register_instruction(nop, overwrite=True)
                        out.append(nop)
                    inst.sync_info = bass_rust.SyncInfo(
                        on_wait=chunks[-1], on_update=ups
                    )
                out.append(inst)
            if len(out) != len(il):
                il[:] = out


def _build_masks():
    mh0 = np.arange(NH0) % (D - 1)
    mh1 = np.arange(NH1) % (D - 1)
    M1 = (mh0[None, :] <= mh1[:, None]).astype(np.float32)
    M0s, M2s = [], []
    for l in range(L):
        perm = np.arange(D) if l % 2 == 0 else np.arange(D)[::-1]
        M0s.append((perm[None, :] <= mh0[:, None]).astype(np.float32))
        M2s.append((mh1[None, :] < perm[:, None]).astype(np.float32))
    return np.stack(M0s), np.broadcast_to(M1, (L,) + M1.shape).copy(), np.stack(M2s)


PRIO_GROUP = 8


def _PRIO(l, ph, t):
    return ((l * 8 + ph) * NT + t) * 64


def _relu(nc, eng, out_ap, in_ap, bias_ap=None):
    AF = mybir.ActivationFunctionType
    ALU = mybir.AluOpType
    if eng == "split":
        c = RELU_SPLIT_C
        _relu(nc, "act", out_ap[:, 0:c], in_ap[:, 0:c], bias_ap)
        _relu(nc, "dve", out_ap[:, c:], in_ap[:, c:], bias_ap)
        return
    if eng == "act":
        nc.scalar.activation(out_ap, in_ap, AF.Relu,
                             bias=0.0 if bias_ap is None else bias_ap)
    else:
        if bias_ap is None:
            nc.vector.tensor_scalar(out_ap, in_ap, 0.0, None, ALU.max)
        else:
            nc.vector.tensor_scalar(out_ap, in_ap, bias_ap, 0.0,
                                    ALU.add, ALU.max)


def _emit_layer(nc, tc, pools, tiles, l, t, last):
    f16, f32, f8 = DT.float16, DT.float32, DT.float8e4
    AF = mybir.ActivationFunctionType
    ALU = mybir.AluOpType
    hpool, lppool, h8pool, epool, opool, pairst = pools
    wt8, wt16, ct, ct8, bt, y16s, y8s = tiles
    c8 = l * CW8
    cb = l * CB
    p, q = t >> 1, t & 1          # pair index, parity (partition half)
    pb = 64 * q                   # partition base of this tile's [64,*] data

    def pair8(off, w):
        return wt8[0:128, c8 + off: c8 + off + 2 * w].rearrange(
            "p (k m) -> p k m", k=2)

    # ph0: mm0 both nets via DoubleRow (y8 | ones) x (A0 | b0row)
    tc.cur_priority = _PRIO(l, 0, t)
    ps0 = {}
    for n in (0, 1):
        pp = hpool.tile([128, 2, BT], f32, tag="hp2", name=f"ps0_{n}")
        for m in (0, 1):
            off = c8 + n * 512 + m * 256
            lhsT = wt8[pb:pb + 64, off: off + 256].rearrange(
                "p (k m) -> p k m", k=2)
            nc.tensor.matmul(pp[:, m, :], lhsT, y8s[p][pb:pb + 64, :, :],
                             start=True, stop=True, perf_mode=DR)
        ps0[n] = pp

    # ph1: relu0, one merged 2-bank op per net (b0 folded via the ones slot)
    tc.cur_priority = _PRIO(l, 1, t)
    h8 = {}
    for n in (0, 1):
        h8[n] = h8pool.tile([128, 2, BT], f8, tag="h8", name=f"h8_{n}")
        _relu(nc, RELU0_ENG[n],
              h8[n][:, :, :].rearrange("p k n -> p (k n)"),
              ps0[n][:, :, :].rearrange("p k n -> p (k n)"))

    # ph2: mm1 DoubleRow K=256, with b1 folded in via a K=1 DoubleRow
    # prebias matmul (b1 row x ones)
    tc.cur_priority = _PRIO(l, 2, t)
    ps1 = {}
    for n in (0, 1):
        pp = hpool.tile([128, 2, BT], f32, tag="hp2", name=f"ps1_{n}")
        for m in (0, 1):
            boff = c8 + 2304 + (n * 2 + m) * 256
            nc.tensor.matmul(pp[:, m, :],
                             wt8[0:1, boff: boff + 256].rearrange(
                                 "p (k m) -> p k m", k=2),
                             ct8[0:1, :, :], start=True, stop=False,
                             perf_mode=DR)
            lhsT = pair8(1024 + n * 512 + m * 256, 128)
            nc.tensor.matmul(pp[:, m, :], lhsT, h8[n][:, :, :],
                             start=False, stop=True, perf_mode=DR)
        ps1[n] = pp

    # ph3: relu1, merged (bias already in psum)
    tc.cur_priority = _PRIO(l, 3, t)
    h18 = {}
    for n in (0, 1):
        h18[n] = h8pool.tile([128, 2, BT], f8, tag="h18", name=f"h18_{n}")
        _relu(nc, RELU1_ENG[n],
              h18[n][:, :, :].rearrange("p k n -> p (k n)"),
              ps1[n][:, :, :].rearrange("p k n -> p (k n)"))

    # ph4: mm2 into the pair's shared psum banks. Even tile -> rows 0:64
    # (DoubleRow), odd tile -> rows 64:128 (plain matmuls: DR can't write
    # partition base 64). loc psum accumulates -b2l (K=1) and +y16 (identity)
    # so it holds t = y - loc - b2l when done.
    tc.cur_priority = _PRIO(l, 4, t)
    if q == 0:
        lp2 = hpool.tile([128, 2, BT], f32, tag="hp2", name="lp2")
        psc, pt = lp2[:, 0, :], lp2[:, 1, :]
        pairst[p] = (psc, pt)
        nc.tensor.matmul(psc[0:64], pair8(2048 + 128, 64), h18[1][:, :, :],
                         start=True, stop=True, perf_mode=DR)
        nc.tensor.matmul(pt[0:64], wt16[0:1, l * CW16: l * CW16 + 64],
                         ct[0:1, 64: 64 + BT], start=True, stop=False)
        nc.tensor.matmul(pt[0:64], pair8(2048, 64), h18[0][:, :, :],
                         start=False, stop=False, perf_mode=DR)
        nc.tensor.matmul(pt[0:64], ct[0:64, 0:64], y16s[p][0:64, :],
                         start=False, stop=True)
        return None
    psc, pt = pairst[p]
    # sc net (plain fp8 matmuls, k-chunks accumulated)
    soff = c8 + 2048 + 128
    for k in (0, 1):
        nc.tensor.matmul(psc[64:128],
                         wt8[0:128, soff + 64 * k: soff + 64 * (k + 1)],
                         h18[1][:, k, :], start=(k == 0), stop=(k == 1))
    # loc net: -b2l, -loc (2 plain chunks), +y16
    loff = c8 + 2048
    nc.tensor.matmul(pt[64:128], wt16[64:65, l * CW16: l * CW16 + 64],
                     ct[64:65, 64: 64 + BT], start=True, stop=False)
    for k in (0, 1):
        nc.tensor.matmul(pt[64:128],
                         wt8[0:128, loff + 64 * k: loff + 64 * (k + 1)],
                         h18[0][:, k, :], start=False, stop=False)
    nc.tensor.matmul(pt[64:128], ct[64:128, 0:64], y16s[p][64:128, :],
                     start=False, stop=True)

    # ph5: paired coupling over both tiles: e = exp(-sc - b2s); y' = t * e
    tc.cur_priority = _PRIO(l, 5, t)
    e16 = epool.tile([128, BT], f16, tag="e")
    nc.scalar.activation(e16[:], psc[:], AF.Exp,
                         bias=bt[:, cb + 4: cb + 5], scale=-1.0)
    if last:
        o32 = opool.tile([128, BT], f32, tag="o32", name="o32")
        nc.vector.tensor_tensor(o32[:], pt[:], e16[:], ALU.mult)
        return o32
    nc.vector.tensor_tensor(y16s[p][:, :], pt[:], e16[:], ALU.mult)
    # fp8 copy for the next layer's mm0 via casting DMA (off the engines)
    nc.gpsimd.dma_start(y8s[p][:, 0, :], y16s[p][:, :])
    return None


def _build():
    _patch_tile_drain(1)
    from contextlib import ExitStack

    f16, f32, f8 = DT.float16, DT.float32, DT.float8e4
    nc = bass.Bass(target_bir_lowering=False)
    u16_d = nc.declare_dram_parameter("u16", [64, BC], f16, isOutput=False)
    u8_d = nc.declare_dram_parameter("u8", [64, BC], f8, isOutput=False)
    w8_d = nc.declare_dram_parameter("w8", [L, 128, CW8], f8, isOutput=False)
    w16_d = nc.declare_dram_parameter("w16", [128, L * CW16], f16, isOutput=False)
    cn_d = nc.declare_dram_parameter("cn", [128, CONST_COLS], f16, isOutput=False)
    b_d = nc.declare_dram_parameter("bias", [128, L * CB], f32, isOutput=False)
    out_d = nc.declare_dram_parameter("out", [64, BC], f32, isOutput=True)

    with tile.TileContext(nc) as tc, ExitStack() as ctx:
        wpool = ctx.enter_context(tc.tile_pool(name="w", bufs=1))
        hpool = ctx.enter_context(tc.tile_pool(name="hp", bufs=4, space="PSUM"))
        lppool = None
        h8pool = ctx.enter_context(tc.tile_pool(name="h8", bufs=8))
        epool = ctx.enter_context(tc.tile_pool(name="e", bufs=6))
        opool = ctx.enter_context(tc.tile_pool(name="o", bufs=3))

        wt8 = wpool.tile([128, L * CW8], f8)
        wt16 = wpool.tile([128, L * CW16], f16)
        ct = wpool.tile([128, CONST_COLS], f16)
        ct8 = wpool.tile([128, 2, BT], f8)      # fp8 ones (prebias moving)
        bt = wpool.tile([128, L * CB], f32)
        nc.gpsimd.memset(ct8[:, :, :], 1.0)

        # PE warmup: keep the clock-ramp monitor busy while DMAs land
        warm = wpool.tile([128, 128], f16)
        wps = hpool.tile([128, 2, WARM_N], f32, tag="hp2", name="wps")
        nc.gpsimd.memset(warm[:], 0.0)
        for _ in range(WARM_MMS):
            nc.tensor.matmul(wps[:, 0, :], warm[:, 0:WARM_N],
                             warm[:, 0:WARM_N], start=True, stop=True)

        nc.sync.dma_start(wt8[:, 0:CW8], w8_d[0])
        nc.sync.dma_start(ct[:], cn_d[:])
        nc.sync.dma_start(wt16[:], w16_d[:])
        nc.sync.dma_start(bt[:], b_d[:])

        y16s, y8s = [], []
        for p in range(NP):
            yt = wpool.tile([128, BT], f16, name=f"y16_{p}")
            nc.sync.dma_start(yt[0:64, :], u16_d[:, (2 * p) * BT:(2 * p + 1) * BT])
            nc.sync.dma_start(yt[64:128, :],
                              u16_d[:, (2 * p + 1) * BT:(2 * p + 2) * BT])
            y16s.append(yt)
        for p in range(NP):
            yt = wpool.tile([128, 2, BT], f8, name=f"y8_{p}")
            nc.sync.dma_start(yt[0:64, 0, :], u8_d[:, (2 * p) * BT:(2 * p + 1) * BT])
            nc.sync.dma_start(yt[64:128, 0, :],
                              u8_d[:, (2 * p + 1) * BT:(2 * p + 2) * BT])
            nc.gpsimd.memset(yt[:, 1, :], 1.0)
            y8s.append(yt)
        for l in range(1, L):
            nc.sync.dma_start(wt8[:, l * CW8:(l + 1) * CW8], w8_d[l])

        pairst = {}
        pools = (hpool, lppool, h8pool, epool, opool, pairst)
        tiles = (wt8, wt16, ct, ct8, bt, y16s, y8s)
        for l in range(L):
            for t in range(NT):
                o32 = _emit_layer(nc, tc, pools, tiles, l, t, l == L - 1)
                if o32 is not None:
                    p = t >> 1
                    nc.sync.dma_start(out_d[:, (2 * p) * BT:(2 * p + 1) * BT],
                                      o32[0:64, :])
                    nc.sync.dma_start(out_d[:, (2 * p + 1) * BT:(2 * p + 2) * BT],
                                      o32[64:128, :])
    _split_excess_waits(nc, maxw=1)
    return nc


_NC_CACHE = None


def _prep_blobs(inputs):
    M0, M1, M2 = _build_masks()
    w8 = np.zeros((L, 128, CW8), F8)
    w16 = np.zeros((128, L * CW16), np.float16)
    cn = np.zeros((128, CONST_COLS), np.float16)
    bb = np.zeros((128, L * CB), np.float32)
    cn[0:64, 0:64] = np.eye(64, dtype=np.float16)
    cn[64:128, 0:64] = np.eye(64, dtype=np.float16)
    cn[:, 64:CONST_COLS] = 1.0
    for l in range(L):
        for n, name in ((0, "loc"), (1, "scale")):
            A0 = (M0[l] * inputs[f"{name}_W0"][l]).astype(np.float32).T  # [64,256]
            A1 = (M1[l] * inputs[f"{name}_W1"][l]).astype(np.float32).T  # [256,256]
            A2 = (M2[l] * inputs[f"{name}_W2"][l]).astype(np.float32).T  # [256,64]
            b0 = inputs[f"{name}_b0"][l].astype(np.float32)
            b1 = inputs[f"{name}_b1"][l].astype(np.float32)
            b2 = inputs[f"{name}_b2"][l].astype(np.float32)
            if n == 0:
                A2 = -A2
                w16[0, l * CW16: l * CW16 + 64] = -b2.astype(np.float16)
                w16[64, l * CW16: l * CW16 + 64] = -b2.astype(np.float16)
            else:
                bb[0:64, l * CB + 4] = -b2
                bb[64:128, l * CB + 4] = -b2
            for m in (0, 1):
                off = n * 512 + m * 256
                a0c = A0[:, m * 128:(m + 1) * 128].astype(F8)
                w8[l, 0:64, off: off + 128] = a0c
                w8[l, 64:128, off: off + 128] = a0c
                b0c = b0[m * 128:(m + 1) * 128].astype(F8)
                w8[l, 0, off + 128: off + 256] = b0c
                w8[l, 64, off + 128: off + 256] = b0c
                off = 1024 + n * 512 + m * 256
                w8[l, :, off: off + 128] = \
                    A1[0:128, m * 128:(m + 1) * 128].astype(F8)
                w8[l, :, off + 128: off + 256] = \
                    A1[128:256, m * 128:(m + 1) * 128].astype(F8)
                bb[:, l * CB + n * 2 + m] = b1[m * 128:(m + 1) * 128]
                boff = 2304 + (n * 2 + m) * 256
                w8[l, 0, boff: boff + 128] = \
                    b1[m * 128:(m + 1) * 128].astype(F8)
            off = 2048 + n * 128
            w8[l, :, off: off + 64] = A2[0:128, :].astype(F8)
            w8[l, :, off + 64: off + 128] = A2[128:256, :].astype(F8)
    return w8, w16, cn, bb


def make_in_maps(inputs):
    inputs = {k: np.asarray(v) for k, v in inputs.items()}
    u = inputs["u"].astype(np.float32)            # [B, 64]
    w8, w16, cn, bb = _prep_blobs(inputs)
    uT16 = np.ascontiguousarray(u.T).astype(np.float16)
    uT8 = uT16.astype(F8)
    in_maps = []
    for c in range(NCORES):
        sl = slice(c * BC, (c + 1) * BC)
        in_maps.append({
            "u16": np.ascontiguousarray(uT16[:, sl]),
            "u8": np.ascontiguousarray(uT8[:, sl]),
            "w8": w8, "w16": w16, "cn": cn, "bias": bb,
        })
    return in_maps


def kernel(**inputs):
    global _NC_CACHE
    if _NC_CACHE is None:
        _NC_CACHE = _build()
    nc = _NC_CACHE
    in_maps = make_in_maps(inputs)
    res = run_bass_kernel_spmd(nc, in_maps, core_ids=list(range(NCORES)))
    out = np.empty((64, B), np.float32)
    for c in range(NCORES):
        out[:, c * BC:(c + 1) * BC] = res.results[c]["out"]
    return np.ascontiguousarray(out.T)


# revision 20
# speedup vs baseline: 1.0045x; 1.0045x over previous
import sys

for _p in ("/opt/trn_rl_repo",):
    if _p not in sys.path:
        sys.path.insert(0, _p)

import numpy as np
import ml_dtypes
import bass_rust
import concourse.bass as bass
import concourse.mybir as mybir
import concourse.tile as tile
from concourse.bass_utils import run_bass_kernel_spmd

DT = mybir.dt
F8 = ml_dtypes.float8_e4m3
DR = mybir.MatmulPerfMode.DoubleRow

# Problem constants (hardcoded from the nn_AutoFlow spec)
B, D, NH0, NH1, L = 32768, 64, 256, 256, 16
NCORES = 8
BC = B // NCORES          # 4096 samples per core
BT = 512                  # batch tile (free dim of activation tiles)
NT = BC // BT             # tiles per core
NP = NT // 2              # tile pairs: tile 2p -> partitions 0:64, 2p+1 -> 64:128
WARM_MMS = 40
WARM_N = 128

# fp8 weight blob column layout, per layer (DoubleRow pairs flattened as
# (k m) so rearrange("p (k m) -> p k m", k=2) recovers the pair)
# mm0 net n chunk m: [64p, 2x128] = (A0 m-chunk | b0 row) at n*512 + m*256
#   (rows 64:128 hold a copy for odd tiles whose y sits at partitions 64:128)
# mm1 net n chunk m: [128p, 2x128] = (A1 k0 mcols | A1 k1) at 1024+n*512+m*256
# mm2 net n:         [128p, 2x64]  = (A2 k0 | A2 k1)       at 2048 + n*128
# b1 prebias (n,m):   [1p, 2x128]  = (b1 chunk | zeros)     at 2304 + (n*2+m)*256
CW8 = 3328

# fp16 blob: per layer col 0:64 = -b2l row (partitions 0 and 64)
CW16 = 64

# const fp16 blob [128, 576]: cols 0:64 = I64 (rows 0:64 and 64:128),
# cols 64:576 = 1.0
CONST_COLS = 576

# fp32 bias blob, per layer 5 cols:
# 0: b1_loc[0:128]  1: b1_loc[128:256]  2: b1_sc[0:128]  3: b1_sc[128:256]
# 4: -b2s (rows 0:64 and 64:128)
CB = 5

# engines for the per-(layer,tile) psum-draining ops. Only ACT ("act") and
# DVE ("dve") may touch PSUM; gpsimd/Pool is SBUF-only on this stack.
RELU0_ENG = {0: "act", 1: "split"}
RELU1_ENG = {0: "act", 1: "dve"}
RELU_SPLIT_C = 96    # "split": flattened cols [0:C] on ACT, [C:2*BT] on DVE
HBT = BT // 2        # coupling psum chunk width


def _patch_tile_drain(maxw=1):
    """walrus on this stack allows only 1 sync-wait on the kernel-tail Drain;
    split the TileContext drain's waits across a chain of drains."""
    from concourse.tile import ScopedClock

    def _drain_and_barrier(self, tick_clock, wait_clock):
        drain_inst = self.nc.sync.drain()
        wait_clock.add_sem_waits(
            drain_inst.ins, ScopedClock({None: tick_clock.global_clock})
        )
        inst = drain_inst.ins
        si = inst.sync_info
        if si is not None:
            waits = list(si.on_wait)
            ups = list(si.on_update)
            if len(waits) > maxw:
                chunks = [waits[i:i + maxw] for i in range(0, len(waits), maxw)]
                inst.sync_info = bass_rust.SyncInfo(on_wait=chunks[0], on_update=[])
                for j, chunk in enumerate(chunks[1:]):
                    extra = self.nc.sync.drain().ins
                    is_last = j == len(chunks) - 2
                    extra.sync_info = bass_rust.SyncInfo(
                        on_wait=chunk, on_update=ups if is_last else []
                    )
        self.nc.all_engine_barrier()
        assert self.sems is not None
        popped = self.nc._tile_sem_poison_stack.pop()
        assert popped is self._sem_poison
        self.nc.clear_and_free_semaphores(list(self.sems.allocated().values()))
        self.nc.all_engine_barrier()

    tile.TileContext._drain_and_barrier = _drain_and_barrier


_MAXW1_TYPES = ("InstDrain", "InstActivation")


def _split_excess_waits(nc, maxw=1):
    """walrus on this stack encodes very few semaphore-wait slots per
    instruction. Spill excess waits onto same-engine NoOps inserted just
    before the instruction (engine streams are in-order, so this is
    equivalent)."""
    for f in nc.m.functions:
        for bb in f.blocks:
            il = bb.instructions
            out = []
            for inst in il:
                si = getattr(inst, "sync_info", None)
                mw = 1 if type(inst).__name__ in _MAXW1_TYPES else maxw
                if si is not None and len(si.on_wait) > mw:
                    waits = list(si.on_wait)
                    ups = list(si.on_update)
                    chunks = [waits[i:i + mw] for i in range(0, len(waits), mw)]
                    for j, ch in enumerate(chunks[:-1]):
                        nop = mybir.InstNoOp(
                            name=f"{inst.name}-wsp{j}", ins=[], outs=[]
                        )
                        nop.engine = inst.engine
                        nop.sync_info = bass_rust.SyncInfo(on_wait=ch, on_update=[])
                        nc.register_instruction(nop, overwrite=True)
                        out.append(nop)
                    inst.sync_info = bass_rust.SyncInfo(
                        on_wait=chunks[-1], on_update=ups
                    )
                out.append(inst)
            if len(out) != len(il):
                il[:] = out


def _build_masks():
    mh0 = np.arange(NH0) % (D - 1)
    mh1 = np.arange(NH1) % (D - 1)
    M1 = (mh0[None, :] <= mh1[:, None]).astype(np.float32)
    M0s, M2s = [], []
    for l in range(L):
        perm = np.arange(D) if l % 2 == 0 else np.arange(D)[::-1]
        M0s.append((perm[None, :] <= mh0[:, None]).astype(np.float32))
        M2s.append((mh1[None, :] < perm[:, None]).astype(np.float32))
    return np.stack(M0s), np.broadcast_to(M1, (L,) + M1.shape).copy(), np.stack(M2s)


PRIO_GROUP = 8


def _PRIO(l, ph, t):
    return ((l * 8 + ph) * NT + t) * 64


def _relu(nc, eng, out_ap, in_ap, bias_ap=None):
    AF = mybir.ActivationFunctionType
    ALU = mybir.AluOpType
    if eng == "split":
        c = RELU_SPLIT_C
        _relu(nc, "act", out_ap[:, 0:c], in_ap[:, 0:c], bias_ap)
        _relu(nc, "dve", out_ap[:, c:], in_ap[:, c:], bias_ap)
        return
    if eng == "act":
        nc.scalar.activation(out_ap, in_ap, AF.Relu,
                             bias=0.0 if bias_ap is None else bias_ap)
    else:
        if bias_ap is None:
            nc.vector.tensor_scalar(out_ap, in_ap, 0.0, None, ALU.max)
        else:
            nc.vector.tensor_scalar(out_ap, in_ap, bias_ap, 0.0,
                                    ALU.add, ALU.max)


def _emit_layer(nc, tc, pools, tiles, l, t, last):
    f16, f32, f8 = DT.float16, DT.float32, DT.float8e4
    AF = mybir.ActivationFunctionType
    ALU = mybir.AluOpType
    hpool, lppool, h8pool, epool, opool, pairst = pools
    wt8, wt16, ct, ct8, bt, y16s, y8s = tiles
    c8 = l * CW8
    cb = l * CB
    p, q = t >> 1, t & 1          # pair index, parity (partition half)
    pb = 64 * q                   # partition base of this tile's [64,*] data

    def pair8(off, w):
        return wt8[0:128, c8 + off: c8 + off + 2 * w].rearrange(
            "p (k m) -> p k m", k=2)

    # ph0: mm0 both nets via DoubleRow (y8 | ones) x (A0 | b0row)
    tc.cur_priority = _PRIO(l, 0, t)
    ps0 = {}
    for n in (0, 1):
        pp = hpool.tile([128, 2, BT], f32, tag="hp2a", bufs=2, name=f"ps0_{n}")
        for m in (0, 1):
            off = c8 + n * 512 + m * 256
            lhsT = wt8[pb:pb + 64, off: off + 256].rearrange(
                "p (k m) -> p k m", k=2)
            nc.tensor.matmul(pp[:, m, :], lhsT, y8s[p][pb:pb + 64, :, :],
                             start=True, stop=True, perf_mode=DR)
        ps0[n] = pp

    # ph1: relu0, one merged 2-bank op per net (b0 folded via the ones slot)
    tc.cur_priority = _PRIO(l, 1, t)
    h8 = {}
    for n in (0, 1):
        h8[n] = h8pool.tile([128, 2, BT], f8, tag="h8", name=f"h8_{n}")
        _relu(nc, RELU0_ENG[n],
              h8[n][:, :, :].rearrange("p k n -> p (k n)"),
              ps0[n][:, :, :].rearrange("p k n -> p (k n)"))

    # ph2: mm1 DoubleRow K=256, with b1 folded in via a K=1 DoubleRow
    # prebias matmul (b1 row x ones)
    tc.cur_priority = _PRIO(l, 2, t)
    ps1 = {}
    for n in (0, 1):
        pp = hpool.tile([128, 2, BT], f32, tag="hp2b", bufs=1, name=f"ps1_{n}")
        for m in (0, 1):
            boff = c8 + 2304 + (n * 2 + m) * 256
            nc.tensor.matmul(pp[:, m, :],
                             wt8[0:1, boff: boff + 256].rearrange(
                                 "p (k m) -> p k m", k=2),
                             ct8[0:1, :, :], start=True, stop=False,
                             perf_mode=DR)
            lhsT = pair8(1024 + n * 512 + m * 256, 128)
            nc.tensor.matmul(pp[:, m, :], lhsT, h8[n][:, :, :],
                             start=False, stop=True, perf_mode=DR)
        ps1[n] = pp

    # ph3: relu1, merged (bias already in psum)
    tc.cur_priority = _PRIO(l, 3, t)
    h18 = {}
    for n in (0, 1):
        h18[n] = h8pool.tile([128, 2, BT], f8, tag="h18", name=f"h18_{n}")
        _relu(nc, RELU1_ENG[n],
              h18[n][:, :, :].rearrange("p k n -> p (k n)"),
              ps1[n][:, :, :].rearrange("p k n -> p (k n)"))

    # ph4: mm2 into the pair's shared psum banks. Even tile -> rows 0:64
    # (DoubleRow), odd tile -> rows 64:128 (plain matmuls: DR can't write
    # partition base 64). loc psum accumulates -b2l (K=1) and +y16 (identity)
    # so it holds t = y - loc - b2l when done.
    tc.cur_priority = _PRIO(l, 4, t)
    if q == 0:
        psc = lppool.tile([128, BT], f32, tag="lsc")
        pt = lppool.tile([128, BT], f32, tag="lt")
        pairst[p] = (psc, pt)
        nc.tensor.matmul(psc[0:64], pair8(2048 + 128, 64), h18[1][:, :, :],
                         start=True, stop=True, perf_mode=DR)
        nc.tensor.matmul(pt[0:64], wt16[0:1, l * CW16: l * CW16 + 64],
                         ct[0:1, 64: 64 + BT], start=True, stop=False)
        nc.tensor.matmul(pt[0:64], pair8(2048, 64), h18[0][:, :, :],
                         start=False, stop=False, perf_mode=DR)
        nc.tensor.matmul(pt[0:64], ct[0:64, 0:64], y16s[p][0:64, :],
                         start=False, stop=True)
        return None
    psc, pt = pairst[p]
    # sc net (plain fp8 matmuls, k-chunks accumulated)
    soff = c8 + 2048 + 128
    for k in (0, 1):
        nc.tensor.matmul(psc[64:128],
                         wt8[0:128, soff + 64 * k: soff + 64 * (k + 1)],
                         h18[1][:, k, :], start=(k == 0), stop=(k == 1))
    # loc net: -b2l, -loc (2 plain chunks), +y16
    loff = c8 + 2048
    nc.tensor.matmul(pt[64:128], wt16[64:65, l * CW16: l * CW16 + 64],
                     ct[64:65, 64: 64 + BT], start=True, stop=False)
    for k in (0, 1):
        nc.tensor.matmul(pt[64:128],
                         wt8[0:128, loff + 64 * k: loff + 64 * (k + 1)],
                         h18[0][:, k, :], start=False, stop=False)
    nc.tensor.matmul(pt[64:128], ct[64:128, 0:64], y16s[p][64:128, :],
                     start=False, stop=True)

    # ph5: paired coupling over both tiles: e = exp(-sc - b2s); y' = t * e
    tc.cur_priority = _PRIO(l, 5, t)
    e16 = epool.tile([128, BT], f16, tag="e")
    nc.scalar.activation(e16[:], psc[:], AF.Exp,
                         bias=bt[:, cb + 4: cb + 5], scale=-1.0)
    if last:
        o32 = opool.tile([128, BT], f32, tag="o32", name="o32")
        nc.vector.tensor_tensor(o32[:], pt[:], e16[:], ALU.mult)
        return o32
    nc.vector.tensor_tensor(y16s[p][:, :], pt[:], e16[:], ALU.mult)
    # fp8 copy for the next layer's mm0 via casting DMA (off the engines)
    nc.gpsimd.dma_start(y8s[p][:, 0, :], y16s[p][:, :])
    return None


def _build():
    _patch_tile_drain(1)
    from contextlib import ExitStack

    f16, f32, f8 = DT.float16, DT.float32, DT.float8e4
    nc = bass.Bass(target_bir_lowering=False)
    u16_d = nc.declare_dram_parameter("u16", [64, BC], f16, isOutput=False)
    u8_d = nc.declare_dram_parameter("u8", [64, BC], f8, isOutput=False)
    w8_d = nc.declare_dram_parameter("w8", [L, 128, CW8], f8, isOutput=False)
    w16_d = nc.declare_dram_parameter("w16", [128, L * CW16], f16, isOutput=False)
    cn_d = nc.declare_dram_parameter("cn", [128, CONST_COLS], f16, isOutput=False)
    b_d = nc.declare_dram_parameter("bias", [128, L * CB], f32, isOutput=False)
    out_d = nc.declare_dram_parameter("out", [64, BC], f32, isOutput=True)

    with tile.TileContext(nc) as tc, ExitStack() as ctx:
        wpool = ctx.enter_context(tc.tile_pool(name="w", bufs=1))
        hpool = ctx.enter_context(tc.tile_pool(name="hp", bufs=3, space="PSUM"))
        lppool = ctx.enter_context(tc.tile_pool(name="lp", bufs=1, space="PSUM"))
        h8pool = ctx.enter_context(tc.tile_pool(name="h8", bufs=8))
        epool = ctx.enter_context(tc.tile_pool(name="e", bufs=6))
        opool = ctx.enter_context(tc.tile_pool(name="o", bufs=3))

        wt8 = wpool.tile([128, L * CW8], f8)
        wt16 = wpool.tile([128, L * CW16], f16)
        ct = wpool.tile([128, CONST_COLS], f16)
        ct8 = wpool.tile([128, 2, BT], f8)      # fp8 ones (prebias moving)
        bt = wpool.tile([128, L * CB], f32)
        nc.gpsimd.memset(ct8[:, :, :], 1.0)

        # PE warmup: keep the clock-ramp monitor busy while DMAs land
        warm = wpool.tile([128, 128], f16)
        wps = lppool.tile([128, WARM_N], f32, tag="lsc", name="wps")
        nc.gpsimd.memset(warm[:], 0.0)
        for _ in range(WARM_MMS):
            nc.tensor.matmul(wps[:, 0:WARM_N], warm[:, 0:WARM_N],
                             warm[:, 0:WARM_N], start=True, stop=True)

        nc.sync.dma_start(wt8[:, 0:CW8], w8_d[0])
        nc.sync.dma_start(ct[:], cn_d[:])
        nc.sync.dma_start(wt16[:], w16_d[:])
        nc.sync.dma_start(bt[:], b_d[:])

        y16s, y8s = [], []
        for p in range(NP):
            yt = wpool.tile([128, BT], f16, name=f"y16_{p}")
            nc.sync.dma_start(yt[0:64, :], u16_d[:, (2 * p) * BT:(2 * p + 1) * BT])
            nc.sync.dma_start(yt[64:128, :],
                              u16_d[:, (2 * p + 1) * BT:(2 * p + 2) * BT])
            y16s.append(yt)
        for p in range(NP):
            yt = wpool.tile([128, 2, BT], f8, name=f"y8_{p}")
            nc.sync.dma_start(yt[0:64, 0, :], u8_d[:, (2 * p) * BT:(2 * p + 1) * BT])
            nc.sync.dma_start(yt[64:128, 0, :],
                              u8_d[:, (2 * p + 1) * BT:(2 * p + 2) * BT])
            nc.gpsimd.memset(yt[:, 1, :], 1.0)
            y8s.append(yt)
        for l in range(1, L):
            nc.sync.dma_start(wt8[:, l * CW8:(l + 1) * CW8], w8_d[l])

        pairst = {}
        pools = (hpool, lppool, h8pool, epool, opool, pairst)
        tiles = (wt8, wt16, ct, ct8, bt, y16s, y8s)
        for l in range(L):
            for t in range(NT):
                o32 = _emit_layer(nc, tc, pools, tiles, l, t, l == L - 1)
                if o32 is not None:
                    p = t >> 1
                    nc.sync.dma_start(out_d[:, (2 * p) * BT:(2 * p + 1) * BT],
                                      o32[0:64, :])
                    nc.sync.dma_start(out_d[:, (2 * p + 1) * BT:(2 * p + 2) * BT],
                                      o32[64:128, :])
    _split_excess_waits(nc, maxw=1)
    return nc


_NC_CACHE = None


def _prep_blobs(inputs):
    M0, M1, M2 = _build_masks()
    w8 = np.zeros((L, 128, CW8), F8)
    w16 = np.zeros((128, L * CW16), np.float16)
    cn = np.zeros((128, CONST_COLS), np.float16)
    bb = np.zeros((128, L * CB), np.float32)
    cn[0:64, 0:64] = np.eye(64, dtype=np.float16)
    cn[64:128, 0:64] = np.eye(64, dtype=np.float16)
    cn[:, 64:CONST_COLS] = 1.0
    for l in range(L):
        for n, name in ((0, "loc"), (1, "scale")):
            A0 = (M0[l] * inputs[f"{name}_W0"][l]).astype(np.float32).T  # [64,256]
            A1 = (M1[l] * inputs[f"{name}_W1"][l]).astype(np.float32).T  # [256,256]
            A2 = (M2[l] * inputs[f"{name}_W2"][l]).astype(np.float32).T  # [256,64]
            b0 = inputs[f"{name}_b0"][l].astype(np.float32)
            b1 = inputs[f"{name}_b1"][l].astype(np.float32)
            b2 = inputs[f"{name}_b2"][l].astype(np.float32)
            if n == 0:
                A2 = -A2
                w16[0, l * CW16: l * CW16 + 64] = -b2.astype(np.float16)
                w16[64, l * CW16: l * CW16 + 64] = -b2.astype(np.float16)
            else:
                bb[0:64, l * CB + 4] = -b2
                bb[64:128, l * CB + 4] = -b2
            for m in (0, 1):
                off = n * 512 + m * 256
                a0c = A0[:, m * 128:(m + 1) * 128].astype(F8)
                w8[l, 0:64, off: off + 128] = a0c
                w8[l, 64:128, off: off + 128] = a0c
                b0c = b0[m * 128:(m + 1) * 128].astype(F8)
                w8[l, 0, off + 128: off + 256] = b0c
                w8[l, 64, off + 128: off + 256] = b0c
                off = 1024 + n * 512 + m * 256
                w8[l, :, off: off + 128] = \
                    A1[0:128, m * 128:(m + 1) * 128].astype(F8)
                w8[l, :, off + 128: off + 256] = \
                    A1[128:256, m * 128:(m + 1) * 128].astype(F8)
                bb[:, l * CB + n * 2 + m] = b1[m * 128:(m + 1) * 128]
                boff = 2304 + (n * 2 + m) * 256
                w8[l, 0, boff: boff + 128] = \
                    b1[m * 128:(m + 1) * 128].astype(F8)
            off = 2048 + n * 128
            w8[l, :, off: off + 64] = A2[0:128, :].astype(F8)
            w8[l, :, off + 64: off + 128] = A2[128:256, :].astype(F8)
    return w8, w16, cn, bb


def make_in_maps(inputs):
    inputs = {k: np.asarray(v) for k, v in inputs.items()}
    u = inputs["u"].astype(np.float32)            # [B, 64]
    w8, w16, cn, bb = _prep_blobs(inputs)
    uT16 = np.ascontiguousarray(u.T).astype(np.float16)
    uT8 = uT16.astype(F8)
    in_maps = []
    for c in range(NCORES):
        sl = slice(c * BC, (c + 1) * BC)
        in_maps.append({
            "u16": np.ascontiguousarray(uT16[:, sl]),
            "u8": np.ascontiguousarray(uT8[:, sl]),
            "w8": w8, "w16": w16, "cn": cn, "bias": bb,
        })
    return in_maps


def kernel(**inputs):
    global _NC_CACHE
    if _NC_CACHE is None:
        _NC_CACHE = _build()
    nc = _NC_CACHE
    in_maps = make_in_maps(inputs)
    res = run_bass_kernel_spmd(nc, in_maps, core_ids=list(range(NCORES)))
    out = np.empty((64, B), np.float32)
    for c in range(NCORES):
        out[:, c * BC:(c + 1) * BC] = res.results[c]["out"]
    return np.ascontiguousarray(out.T)


# revision 23
# speedup vs baseline: 1.4739x; 1.4673x over previous
import sys

for _p in ("/opt/trn_rl_repo",):
    if _p not in sys.path:
        sys.path.insert(0, _p)

import numpy as np
import ml_dtypes
import bass_rust
import concourse.bass as bass
import concourse.mybir as mybir
import concourse.tile as tile
from concourse.bass_utils import run_bass_kernel_spmd

DT = mybir.dt
F8 = ml_dtypes.float8_e4m3
DR = mybir.MatmulPerfMode.DoubleRow

# Problem constants (hardcoded from the nn_AutoFlow spec)
B, D, NH0, NH1, L = 32768, 64, 256, 256, 16
NCORES = 8
BC = B // NCORES          # 4096 samples per core
BT = 512                  # batch tile (free dim of activation tiles)
NT = BC // BT             # tiles per core
NP = NT // 2              # tile pairs: tile 2p -> partitions 0:64, 2p+1 -> 64:128
WARM_MMS = 40
WARM_N = 128

# fp8 weight blob column layout, per layer (DoubleRow pairs flattened as
# (k m) so rearrange("p (k m) -> p k m", k=2) recovers the pair)
# mm0 net n chunk m: [64p, 2x128] = (A0 m-chunk | b0 row) at n*512 + m*256
#   (rows 64:128 hold a copy for odd tiles whose y sits at partitions 64:128)
# mm1 net n chunk m: [128p, 2x128] = (A1 k0 mcols | A1 k1) at 1024+n*512+m*256
# mm2 net n:         [128p, 2x64]  = (A2 k0 | A2 k1)       at 2048 + n*128
# b1 prebias (n,m):   [1p, 2x128]  = (b1 chunk | zeros)     at 2304 + (n*2+m)*256
CW8 = 3328

# fp16 blob: per layer col 0:64 = -b2l row (partitions 0 and 64)
CW16 = 64

# const fp16 blob [128, 576]: cols 0:64 = I64 (rows 0:64 and 64:128),
# cols 64:576 = 1.0
CONST_COLS = 576

# fp32 bias blob, per layer 5 cols:
# 0: b1_loc[0:128]  1: b1_loc[128:256]  2: b1_sc[0:128]  3: b1_sc[128:256]
# 4: -b2s (rows 0:64 and 64:128)
CB = 5

# engines for the per-(layer,tile) psum-draining ops. Only ACT ("act") and
# DVE ("dve") may touch PSUM; gpsimd/Pool is SBUF-only on this stack.
CFG = {
    "merge": False,           # merged 2-bank relus + b1 DR prebias
    "relu0": {0: "act", 1: "split"},
    "relu1": {0: "act", 1: "dve"},
    # unmerged per-(net,m) maps: interleave engines across m-chunks so both
    # engines drain a net's two psum banks in parallel
    "relu0u": {(0, 0): "act", (0, 1): "dve", (1, 0): "act", (1, 1): "dve"},
    "relu1u": {(0, 0): "dve", (0, 1): "act", (1, 0): "dve", (1, 1): "act"},
    "split_c": 135,           # "split": flattened cols [0:C] on ACT, rest DVE
    "prio": "pair",           # phase | tile | skew | pair
    "hp_bufs": 3,             # merged: [128,2,BT] tiles; unmerged: [128,BT]
    "hp_bufs_u": 6,
    "skew_off": (0, 3, 9, 12, 17, 18),
    # (phase, n, m, mod, rems, eng): for tiles with t % mod in rems, run
    # relu<phase> of chunk (n, m) on `eng` instead of the mapped engine
    "flips": [],
}


def _patch_tile_drain(maxw=1):
    """walrus on this stack allows only 1 sync-wait on the kernel-tail Drain;
    split the TileContext drain's waits across a chain of drains."""
    from concourse.tile import ScopedClock

    def _drain_and_barrier(self, tick_clock, wait_clock):
        drain_inst = self.nc.sync.drain()
        wait_clock.add_sem_waits(
            drain_inst.ins, ScopedClock({None: tick_clock.global_clock})
        )
        inst = drain_inst.ins
        si = inst.sync_info
        if si is not None:
            waits = list(si.on_wait)
            ups = list(si.on_update)
            if len(waits) > maxw:
                chunks = [waits[i:i + maxw] for i in range(0, len(waits), maxw)]
                inst.sync_info = bass_rust.SyncInfo(on_wait=chunks[0], on_update=[])
                for j, chunk in enumerate(chunks[1:]):
                    extra = self.nc.sync.drain().ins
                    is_last = j == len(chunks) - 2
                    extra.sync_info = bass_rust.SyncInfo(
                        on_wait=chunk, on_update=ups if is_last else []
                    )
        self.nc.all_engine_barrier()
        assert self.sems is not None
        popped = self.nc._tile_sem_poison_stack.pop()
        assert popped is self._sem_poison
        self.nc.clear_and_free_semaphores(list(self.sems.allocated().values()))
        self.nc.all_engine_barrier()

    tile.TileContext._drain_and_barrier = _drain_and_barrier


_MAXW1_TYPES = ("InstDrain", "InstActivation")


def _split_excess_waits(nc, maxw=1):
    """walrus on this stack encodes very few semaphore-wait slots per
    instruction. Spill excess waits onto same-engine NoOps inserted just
    before the instruction (engine streams are in-order, so this is
    equivalent)."""
    for f in nc.m.functions:
        for bb in f.blocks:
            il = bb.instructions
            out = []
            for inst in il:
                si = getattr(inst, "sync_info", None)
                mw = 1 if type(inst).__name__ in _MAXW1_TYPES else maxw
                if si is not None and len(si.on_wait) > mw:
                    waits = list(si.on_wait)
                    ups = list(si.on_update)
                    chunks = [waits[i:i + mw] for i in range(0, len(waits), mw)]
                    for j, ch in enumerate(chunks[:-1]):
                        nop = mybir.InstNoOp(
                            name=f"{inst.name}-wsp{j}", ins=[], outs=[]
                        )
                        nop.engine = inst.engine
                        nop.sync_info = bass_rust.SyncInfo(on_wait=ch, on_update=[])
                        nc.register_instruction(nop, overwrite=True)
                        out.append(nop)
                    inst.sync_info = bass_rust.SyncInfo(
                        on_wait=chunks[-1], on_update=ups
                    )
                out.append(inst)
            if len(out) != len(il):
                il[:] = out


def _build_masks():
    mh0 = np.arange(NH0) % (D - 1)
    mh1 = np.arange(NH1) % (D - 1)
    M1 = (mh0[None, :] <= mh1[:, None]).astype(np.float32)
    M0s, M2s = [], []
    for l in range(L):
        perm = np.arange(D) if l % 2 == 0 else np.arange(D)[::-1]
        M0s.append((perm[None, :] <= mh0[:, None]).astype(np.float32))
        M2s.append((mh1[None, :] < perm[:, None]).astype(np.float32))
    return np.stack(M0s), np.broadcast_to(M1, (L,) + M1.shape).copy(), np.stack(M2s)


PRIO_GROUP = 8


def _PRIO(l, ph, t):
    mode = CFG["prio"]
    if mode == "phase":
        return ((l * 8 + ph) * NT + t) * 64
    if mode == "tile":
        return ((l * NT + t) * 8 + ph) * 64
    if mode == "skew":
        return ((l * NT + t) * 8 + CFG["skew_off"][ph]) * 64
    if mode == "pair":
        return ((((l * NP) + (t >> 1)) * 8 + ph) * 2 + (t & 1)) * 64
    raise ValueError(mode)


def _eng(phase, n, m, t):
    for (ph, fn, fm, mod, rems, eng) in CFG["flips"]:
        if ph == phase and fn == n and fm == m and (t % mod) in rems:
            return eng
    return (CFG["relu0u"] if phase == 0 else CFG["relu1u"])[(n, m)]


def _relu(nc, eng, out_ap, in_ap, bias_ap=None):
    AF = mybir.ActivationFunctionType
    ALU = mybir.AluOpType
    if eng == "split":
        c = CFG["split_c"]
        _relu(nc, "act", out_ap[:, 0:c], in_ap[:, 0:c], bias_ap)
        _relu(nc, "dve", out_ap[:, c:], in_ap[:, c:], bias_ap)
        return
    if eng == "act":
        nc.scalar.activation(out_ap, in_ap, AF.Relu,
                             bias=0.0 if bias_ap is None else bias_ap)
    else:
        if bias_ap is None:
            nc.vector.tensor_scalar(out_ap, in_ap, 0.0, None, ALU.max)
        else:
            nc.vector.tensor_scalar(out_ap, in_ap, bias_ap, 0.0,
                                    ALU.add, ALU.max)


def _emit_layer(nc, tc, pools, tiles, l, t, last):
    f16, f32, f8 = DT.float16, DT.float32, DT.float8e4
    AF = mybir.ActivationFunctionType
    ALU = mybir.AluOpType
    hpool, lppool, h8pool, epool, opool, pairst = pools
    wt8, wt16, ct, ct8, bt, y16s, y8s = tiles
    c8 = l * CW8
    cb = l * CB
    p, q = t >> 1, t & 1          # pair index, parity (partition half)
    pb = 64 * q                   # partition base of this tile's [64,*] data

    def pair8(off, w):
        return wt8[0:128, c8 + off: c8 + off + 2 * w].rearrange(
            "p (k m) -> p k m", k=2)

    # ph0: mm0 both nets via DoubleRow (y8 | ones) x (A0 | b0row)
    tc.cur_priority = _PRIO(l, 0, t)
    merged = CFG["merge"]
    ps0 = {}
    for n in (0, 1):
        if merged:
            pp = hpool.tile([128, 2, BT], f32, tag="hp2", bufs=CFG["hp_bufs"],
                            name=f"ps0_{n}")
        for m in (0, 1):
            if not merged:
                pp = hpool.tile([128, BT], f32, tag="hp",
                                bufs=CFG["hp_bufs_u"], name=f"ps0_{n}{m}")
            off = c8 + n * 512 + m * 256
            lhsT = wt8[pb:pb + 64, off: off + 256].rearrange(
                "p (k m) -> p k m", k=2)
            dst = pp[:, m, :] if merged else pp[:, :]
            nc.tensor.matmul(dst, lhsT, y8s[p][pb:pb + 64, :, :],
                             start=True, stop=True, perf_mode=DR)
            ps0[(n, m)] = pp
        ps0[n] = pp

    # ph1: relu0 (b0 folded via the ones slot)
    tc.cur_priority = _PRIO(l, 1, t)
    h8 = {}
    for n in (0, 1):
        h8[n] = h8pool.tile([128, 2, BT], f8, tag="h8", name=f"h8_{n}")
        if merged:
            _relu(nc, CFG["relu0"][n],
                  h8[n][:, :, :].rearrange("p k n -> p (k n)"),
                  ps0[n][:, :, :].rearrange("p k n -> p (k n)"))
        else:
            for m in (0, 1):
                _relu(nc, _eng(0, n, m, t), h8[n][:, m, :],
                      ps0[(n, m)][:, :])

    # ph2: mm1 DoubleRow K=256 (+ b1 K=1 DR prebias when merged)
    tc.cur_priority = _PRIO(l, 2, t)
    ps1 = {}
    for n in (0, 1):
        if merged:
            pp = hpool.tile([128, 2, BT], f32, tag="hp2", bufs=CFG["hp_bufs"],
                            name=f"ps1_{n}")
        for m in (0, 1):
            if not merged:
                pp = hpool.tile([128, BT], f32, tag="hp",
                                bufs=CFG["hp_bufs_u"], name=f"ps1_{n}{m}")
            dst = pp[:, m, :] if merged else pp[:, :]
            if merged:
                boff = c8 + 2304 + (n * 2 + m) * 256
                nc.tensor.matmul(dst,
                                 wt8[0:1, boff: boff + 256].rearrange(
                                     "p (k m) -> p k m", k=2),
                                 ct8[0:1, :, :], start=True, stop=False,
                                 perf_mode=DR)
            lhsT = pair8(1024 + n * 512 + m * 256, 128)
            nc.tensor.matmul(dst, lhsT, h8[n][:, :, :],
                             start=(not merged), stop=True, perf_mode=DR)
            ps1[(n, m)] = pp
        ps1[n] = pp

    # ph3: relu1 (bias in psum when merged, engine bias otherwise)
    tc.cur_priority = _PRIO(l, 3, t)
    h18 = {}
    for n in (0, 1):
        h18[n] = h8pool.tile([128, 2, BT], f8, tag="h18", name=f"h18_{n}")
        if merged:
            _relu(nc, CFG["relu1"][n],
                  h18[n][:, :, :].rearrange("p k n -> p (k n)"),
                  ps1[n][:, :, :].rearrange("p k n -> p (k n)"))
        else:
            for m in (0, 1):
                bias_ap = bt[:, cb + n * 2 + m: cb + n * 2 + m + 1]
                _relu(nc, _eng(1, n, m, t), h18[n][:, m, :],
                      ps1[(n, m)][:, :], bias_ap)

    # ph4: mm2 into the pair's shared psum banks. Even tile -> rows 0:64
    # (DoubleRow), odd tile -> rows 64:128 (plain matmuls: DR can't write
    # partition base 64). loc psum accumulates -b2l (K=1) and +y16 (identity)
    # so it holds t = y - loc - b2l when done.
    tc.cur_priority = _PRIO(l, 4, t)
    if q == 0:
        psc = lppool.tile([128, BT], f32, tag="lsc")
        pt = lppool.tile([128, BT], f32, tag="lt")
        pairst[p] = (psc, pt)
        nc.tensor.matmul(psc[0:64], pair8(2048 + 128, 64), h18[1][:, :, :],
                         start=True, stop=True, perf_mode=DR)
        nc.tensor.matmul(pt[0:64], wt16[0:1, l * CW16: l * CW16 + 64],
                         ct[0:1, 64: 64 + BT], start=True, stop=False)
        nc.tensor.matmul(pt[0:64], pair8(2048, 64), h18[0][:, :, :],
                         start=False, stop=False, perf_mode=DR)
        nc.tensor.matmul(pt[0:64], ct[0:64, 0:64], y16s[p][0:64, :],
                         start=False, stop=True)
        return None
    psc, pt = pairst[p]
    # sc net (plain fp8 matmuls, k-chunks accumulated)
    soff = c8 + 2048 + 128
    for k in (0, 1):
        nc.tensor.matmul(psc[64:128],
                         wt8[0:128, soff + 64 * k: soff + 64 * (k + 1)],
                         h18[1][:, k, :], start=(k == 0), stop=(k == 1))
    # loc net: -b2l, -loc (2 plain chunks), +y16
    loff = c8 + 2048
    nc.tensor.matmul(pt[64:128], wt16[64:65, l * CW16: l * CW16 + 64],
                     ct[64:65, 64: 64 + BT], start=True, stop=False)
    for k in (0, 1):
        nc.tensor.matmul(pt[64:128],
                         wt8[0:128, loff + 64 * k: loff + 64 * (k + 1)],
                         h18[0][:, k, :], start=False, stop=False)
    nc.tensor.matmul(pt[64:128], ct[64:128, 0:64], y16s[p][64:128, :],
                     start=False, stop=True)

    # ph5: paired coupling over both tiles: e = exp(-sc - b2s); y' = t * e
    tc.cur_priority = _PRIO(l, 5, t)
    e16 = epool.tile([128, BT], f16, tag="e")
    nc.scalar.activation(e16[:], psc[:], AF.Exp,
                         bias=bt[:, cb + 4: cb + 5], scale=-1.0)
    if last:
        o32 = opool.tile([128, BT], f32, tag="o32", name="o32")
        nc.vector.tensor_tensor(o32[:], pt[:], e16[:], ALU.mult)
        return o32
    nc.vector.tensor_tensor(y16s[p][:, :], pt[:], e16[:], ALU.mult)
    # fp8 copy for the next layer's mm0 via casting DMA (off the engines)
    nc.gpsimd.dma_start(y8s[p][:, 0, :], y16s[p][:, :])
    return None


def _build():
    _patch_tile_drain(1)
    from contextlib import ExitStack

    f16, f32, f8 = DT.float16, DT.float32, DT.float8e4
    nc = bass.Bass(target_bir_lowering=False)
    u16_d = nc.declare_dram_parameter("u16", [64, BC], f16, isOutput=False)
    u8_d = nc.declare_dram_parameter("u8", [64, BC], f8, isOutput=False)
    w8_d = nc.declare_dram_parameter("w8", [L, 128, CW8], f8, isOutput=False)
    w16_d = nc.declare_dram_parameter("w16", [128, L * CW16], f16, isOutput=False)
    cn_d = nc.declare_dram_parameter("cn", [128, CONST_COLS], f16, isOutput=False)
    b_d = nc.declare_dram_parameter("bias", [128, L * CB], f32, isOutput=False)
    out_d = nc.declare_dram_parameter("out", [64, BC], f32, isOutput=True)

    with tile.TileContext(nc) as tc, ExitStack() as ctx:
        wpool = ctx.enter_context(tc.tile_pool(name="w", bufs=1))
        hpool = ctx.enter_context(tc.tile_pool(name="hp", bufs=1, space="PSUM"))
        lppool = ctx.enter_context(tc.tile_pool(name="lp", bufs=1, space="PSUM"))
        h8pool = ctx.enter_context(tc.tile_pool(name="h8", bufs=8))
        epool = ctx.enter_context(tc.tile_pool(name="e", bufs=6))
        opool = ctx.enter_context(tc.tile_pool(name="o", bufs=3))

        wt8 = wpool.tile([128, L * CW8], f8)
        wt16 = wpool.tile([128, L * CW16], f16)
        ct = wpool.tile([128, CONST_COLS], f16)
        ct8 = wpool.tile([128, 2, BT], f8)      # fp8 ones (prebias moving)
        bt = wpool.tile([128, L * CB], f32)
        nc.gpsimd.memset(ct8[:, :, :], 1.0)

        # PE warmup: keep the clock-ramp monitor busy while DMAs land
        warm = wpool.tile([128, 128], f16)
        wps = lppool.tile([128, WARM_N], f32, tag="lsc", name="wps")
        nc.gpsimd.memset(warm[:], 0.0)
        for _ in range(WARM_MMS):
            nc.tensor.matmul(wps[:, 0:WARM_N], warm[:, 0:WARM_N],
                             warm[:, 0:WARM_N], start=True, stop=True)

        nc.sync.dma_start(wt8[:, 0:CW8], w8_d[0])
        nc.sync.dma_start(ct[:], cn_d[:])
        nc.sync.dma_start(wt16[:], w16_d[:])
        nc.sync.dma_start(bt[:], b_d[:])

        y16s, y8s = [], []
        for p in range(NP):
            yt = wpool.tile([128, BT], f16, name=f"y16_{p}")
            nc.sync.dma_start(yt[0:64, :], u16_d[:, (2 * p) * BT:(2 * p + 1) * BT])
            nc.sync.dma_start(yt[64:128, :],
                              u16_d[:, (2 * p + 1) * BT:(2 * p + 2) * BT])
            y16s.append(yt)
        for p in range(NP):
            yt = wpool.tile([128, 2, BT], f8, name=f"y8_{p}")
            nc.sync.dma_start(yt[0:64, 0, :], u8_d[:, (2 * p) * BT:(2 * p + 1) * BT])
            nc.sync.dma_start(yt[64:128, 0, :],
                              u8_d[:, (2 * p + 1) * BT:(2 * p + 2) * BT])
            nc.gpsimd.memset(yt[:, 1, :], 1.0)
            y8s.append(yt)
        for l in range(1, L):
            nc.sync.dma_start(wt8[:, l * CW8:(l + 1) * CW8], w8_d[l])

        pairst = {}
        pools = (hpool, lppool, h8pool, epool, opool, pairst)
        tiles = (wt8, wt16, ct, ct8, bt, y16s, y8s)
        for l in range(L):
            for t in range(NT):
                o32 = _emit_layer(nc, tc, pools, tiles, l, t, l == L - 1)
                if o32 is not None:
                    p = t >> 1
                    nc.sync.dma_start(out_d[:, (2 * p) * BT:(2 * p + 1) * BT],
                                      o32[0:64, :])
                    nc.sync.dma_start(out_d[:, (2 * p + 1) * BT:(2 * p + 2) * BT],
                                      o32[64:128, :])
    _split_excess_waits(nc, maxw=1)
    return nc


_NC_CACHE = None


def _prep_blobs(inputs):
    M0, M1, M2 = _build_masks()
    w8 = np.zeros((L, 128, CW8), F8)
    w16 = np.zeros((128, L * CW16), np.float16)
    cn = np.zeros((128, CONST_COLS), np.float16)
    bb = np.zeros((128, L * CB), np.float32)
    cn[0:64, 0:64] = np.eye(64, dtype=np.float16)
    cn[64:128, 0:64] = np.eye(64, dtype=np.float16)
    cn[:, 64:CONST_COLS] = 1.0
    for l in range(L):
        for n, name in ((0, "loc"), (1, "scale")):
            A0 = (M0[l] * inputs[f"{name}_W0"][l]).astype(np.float32).T  # [64,256]
            A1 = (M1[l] * inputs[f"{name}_W1"][l]).astype(np.float32).T  # [256,256]
            A2 = (M2[l] * inputs[f"{name}_W2"][l]).astype(np.float32).T  # [256,64]
            b0 = inputs[f"{name}_b0"][l].astype(np.float32)
            b1 = inputs[f"{name}_b1"][l].astype(np.float32)
            b2 = inputs[f"{name}_b2"][l].astype(np.float32)
            if n == 0:
                A2 = -A2
                w16[0, l * CW16: l * CW16 + 64] = -b2.astype(np.float16)
                w16[64, l * CW16: l * CW16 + 64] = -b2.astype(np.float16)
            else:
                bb[0:64, l * CB + 4] = -b2
                bb[64:128, l * CB + 4] = -b2
            for m in (0, 1):
                off = n * 512 + m * 256
                a0c = A0[:, m * 128:(m + 1) * 128].astype(F8)
                w8[l, 0:64, off: off + 128] = a0c
                w8[l, 64:128, off: off + 128] = a0c
                b0c = b0[m * 128:(m + 1) * 128].astype(F8)
                w8[l, 0, off + 128: off + 256] = b0c
                w8[l, 64, off + 128: off + 256] = b0c
                off = 1024 + n * 512 + m * 256
                w8[l, :, off: off + 128] = \
                    A1[0:128, m * 128:(m + 1) * 128].astype(F8)
                w8[l, :, off + 128: off + 256] = \
                    A1[128:256, m * 128:(m + 1) * 128].astype(F8)
                bb[:, l * CB + n * 2 + m] = b1[m * 128:(m + 1) * 128]
                boff = 2304 + (n * 2 + m) * 256
                w8[l, 0, boff: boff + 128] = \
                    b1[m * 128:(m + 1) * 128].astype(F8)
            off = 2048 + n * 128
            w8[l, :, off: off + 64] = A2[0:128, :].astype(F8)
            w8[l, :, off + 64: off + 128] = A2[128:256, :].astype(F8)
    return w8, w16, cn, bb


def make_in_maps(inputs):
    inputs = {k: np.asarray(v) for k, v in inputs.items()}
    u = inputs["u"].astype(np.float32)            # [B, 64]
    w8, w16, cn, bb = _prep_blobs(inputs)
    uT16 = np.ascontiguousarray(u.T).astype(np.float16)
    uT8 = uT16.astype(F8)
    in_maps = []
    for c in range(NCORES):
        sl = slice(c * BC, (c + 1) * BC)
        in_maps.append({
            "u16": np.ascontiguousarray(uT16[:, sl]),
            "u8": np.ascontiguousarray(uT8[:, sl]),
            "w8": w8, "w16": w16, "cn": cn, "bias": bb,
        })
    return in_maps


def kernel(**inputs):
    global _NC_CACHE
    if _NC_CACHE is None:
        _NC_CACHE = _build()
    nc = _NC_CACHE
    in_maps = make_in_maps(inputs)
    res = run_bass_kernel_spmd(nc, in_maps, core_ids=list(range(NCORES)))
    out = np.empty((64, B), np.float32)
    for c in range(NCORES):
        out[:, c * BC:(c + 1) * BC] = res.results[c]["out"]
    return np.ascontiguousarray(out.T)


# revision 26
# speedup vs baseline: 1.4826x; 1.0059x over previous
import sys

for _p in ("/opt/trn_rl_repo",):
    if _p not in sys.path:
        sys.path.insert(0, _p)

import numpy as np
import ml_dtypes
import bass_rust
import concourse.bass as bass
import concourse.mybir as mybir
import concourse.tile as tile
from concourse.bass_utils import run_bass_kernel_spmd

DT = mybir.dt
F8 = ml_dtypes.float8_e4m3
DR = mybir.MatmulPerfMode.DoubleRow

# Problem constants (hardcoded from the nn_AutoFlow spec)
B, D, NH0, NH1, L = 32768, 64, 256, 256, 16
NCORES = 8
BC = B // NCORES          # 4096 samples per core
BT = 512                  # batch tile (free dim of activation tiles)
NT = BC // BT             # tiles per core
NP = NT // 2              # tile pairs: tile 2p -> partitions 0:64, 2p+1 -> 64:128
WARM_MMS = 40
WARM_N = 128

# fp8 weight blob column layout, per layer (DoubleRow pairs flattened as
# (k m) so rearrange("p (k m) -> p k m", k=2) recovers the pair)
# mm0 net n chunk m: [64p, 2x128] = (A0 m-chunk | b0 row) at n*512 + m*256
#   (rows 64:128 hold a copy for odd tiles whose y sits at partitions 64:128)
# mm1 net n chunk m: [128p, 2x128] = (A1 k0 mcols | A1 k1) at 1024+n*512+m*256
# mm2 net n:         [128p, 2x64]  = (A2 k0 | A2 k1)       at 2048 + n*128
# b1 prebias (n,m):   [1p, 2x128]  = (b1 chunk | zeros)     at 2304 + (n*2+m)*256
CW8 = 3328

# fp16 blob: per layer col 0:64 = -b2l row (partitions 0 and 64)
CW16 = 64

# const fp16 blob [128, 576]: cols 0:64 = I64 (rows 0:64 and 64:128),
# cols 64:576 = 1.0
CONST_COLS = 576

# fp32 bias blob, per layer 5 cols:
# 0: b1_loc[0:128]  1: b1_loc[128:256]  2: b1_sc[0:128]  3: b1_sc[128:256]
# 4: -b2s (rows 0:64 and 64:128)
CB = 5

# engines for the per-(layer,tile) psum-draining ops. Only ACT ("act") and
# DVE ("dve") may touch PSUM; gpsimd/Pool is SBUF-only on this stack.
CFG = {
    "merge": False,           # merged 2-bank relus + b1 DR prebias
    "relu0": {0: "act", 1: "split"},
    "relu1": {0: "act", 1: "dve"},
    # unmerged per-(net,m) maps: interleave engines across m-chunks so both
    # engines drain a net's two psum banks in parallel
    "relu0u": {(0, 0): "act", (0, 1): "dve", (1, 0): "act", (1, 1): "dve"},
    "relu1u": {(0, 0): "dve", (0, 1): "act", (1, 0): "dve", (1, 1): "act"},
    "split_c": 135,           # "split": flattened cols [0:C] on ACT, rest DVE
    "prio": "pair",           # phase | tile | skew | pair
    "hp_bufs": 3,             # merged: [128,2,BT] tiles; unmerged: [128,BT]
    "hp_bufs_u": 6,
    "skew_off": (0, 3, 9, 12, 17, 18),
    # (phase, n, m, mod, rems, eng): for tiles with t % mod in rems, run
    # relu<phase> of chunk (n, m) on `eng` instead of the mapped engine
    "flips": [(1, 1, 0, 8, (0,), "act")],
    "h8_bufs": 10, "e_bufs": 8, "warm": 40,
}


def _patch_tile_drain(maxw=1):
    """walrus on this stack allows only 1 sync-wait on the kernel-tail Drain;
    split the TileContext drain's waits across a chain of drains."""
    from concourse.tile import ScopedClock

    def _drain_and_barrier(self, tick_clock, wait_clock):
        drain_inst = self.nc.sync.drain()
        wait_clock.add_sem_waits(
            drain_inst.ins, ScopedClock({None: tick_clock.global_clock})
        )
        inst = drain_inst.ins
        si = inst.sync_info
        if si is not None:
            waits = list(si.on_wait)
            ups = list(si.on_update)
            if len(waits) > maxw:
                chunks = [waits[i:i + maxw] for i in range(0, len(waits), maxw)]
                inst.sync_info = bass_rust.SyncInfo(on_wait=chunks[0], on_update=[])
                for j, chunk in enumerate(chunks[1:]):
                    extra = self.nc.sync.drain().ins
                    is_last = j == len(chunks) - 2
                    extra.sync_info = bass_rust.SyncInfo(
                        on_wait=chunk, on_update=ups if is_last else []
                    )
        self.nc.all_engine_barrier()
        assert self.sems is not None
        popped = self.nc._tile_sem_poison_stack.pop()
        assert popped is self._sem_poison
        self.nc.clear_and_free_semaphores(list(self.sems.allocated().values()))
        self.nc.all_engine_barrier()

    tile.TileContext._drain_and_barrier = _drain_and_barrier


_MAXW1_TYPES = ("InstDrain", "InstActivation")


def _split_excess_waits(nc, maxw=1):
    """walrus on this stack encodes very few semaphore-wait slots per
    instruction. Spill excess waits onto same-engine NoOps inserted just
    before the instruction (engine streams are in-order, so this is
    equivalent)."""
    for f in nc.m.functions:
        for bb in f.blocks:
            il = bb.instructions
            out = []
            for inst in il:
                si = getattr(inst, "sync_info", None)
                mw = 1 if type(inst).__name__ in _MAXW1_TYPES else maxw
                if si is not None and len(si.on_wait) > mw:
                    waits = list(si.on_wait)
                    ups = list(si.on_update)
                    chunks = [waits[i:i + mw] for i in range(0, len(waits), mw)]
                    for j, ch in enumerate(chunks[:-1]):
                        nop = mybir.InstNoOp(
                            name=f"{inst.name}-wsp{j}", ins=[], outs=[]
                        )
                        nop.engine = inst.engine
                        nop.sync_info = bass_rust.SyncInfo(on_wait=ch, on_update=[])
                        nc.register_instruction(nop, overwrite=True)
                        out.append(nop)
                    inst.sync_info = bass_rust.SyncInfo(
                        on_wait=chunks[-1], on_update=ups
                    )
                out.append(inst)
            if len(out) != len(il):
                il[:] = out


def _build_masks():
    mh0 = np.arange(NH0) % (D - 1)
    mh1 = np.arange(NH1) % (D - 1)
    M1 = (mh0[None, :] <= mh1[:, None]).astype(np.float32)
    M0s, M2s = [], []
    for l in range(L):
        perm = np.arange(D) if l % 2 == 0 else np.arange(D)[::-1]
        M0s.append((perm[None, :] <= mh0[:, None]).astype(np.float32))
        M2s.append((mh1[None, :] < perm[:, None]).astype(np.float32))
    return np.stack(M0s), np.broadcast_to(M1, (L,) + M1.shape).copy(), np.stack(M2s)


PRIO_GROUP = 8


def _PRIO(l, ph, t):
    mode = CFG["prio"]
    if mode == "phase":
        return ((l * 8 + ph) * NT + t) * 64
    if mode == "tile":
        return ((l * NT + t) * 8 + ph) * 64
    if mode == "skew":
        return ((l * NT + t) * 8 + CFG["skew_off"][ph]) * 64
    if mode == "pair":
        return ((((l * NP) + (t >> 1)) * 8 + ph) * 2 + (t & 1)) * 64
    if mode == "quad":
        return ((((l * (NT // 4)) + (t >> 2)) * 8 + ph) * 4 + (t & 3)) * 64
    if mode == "pairc":
        phx = {0: 0, 1: 1, 2: 2, 3: 3, 4: 4, 5: 4}[ph]
        return ((((l * NP) + (t >> 1)) * 8 + phx) * 2 + (t & 1)) * 64 + ph
    raise ValueError(mode)


def _eng(phase, n, m, t):
    for (ph, fn, fm, mod, rems, eng) in CFG["flips"]:
        if ph == phase and fn == n and fm == m and (t % mod) in rems:
            return eng
    return (CFG["relu0u"] if phase == 0 else CFG["relu1u"])[(n, m)]


def _relu(nc, eng, out_ap, in_ap, bias_ap=None):
    AF = mybir.ActivationFunctionType
    ALU = mybir.AluOpType
    if eng == "split":
        c = CFG["split_c"]
        _relu(nc, "act", out_ap[:, 0:c], in_ap[:, 0:c], bias_ap)
        _relu(nc, "dve", out_ap[:, c:], in_ap[:, c:], bias_ap)
        return
    if eng == "act":
        nc.scalar.activation(out_ap, in_ap, AF.Relu,
                             bias=0.0 if bias_ap is None else bias_ap)
    else:
        if bias_ap is None:
            nc.vector.tensor_scalar(out_ap, in_ap, 0.0, None, ALU.max)
        else:
            nc.vector.tensor_scalar(out_ap, in_ap, bias_ap, 0.0,
                                    ALU.add, ALU.max)


def _emit_layer(nc, tc, pools, tiles, l, t, last):
    f16, f32, f8 = DT.float16, DT.float32, DT.float8e4
    AF = mybir.ActivationFunctionType
    ALU = mybir.AluOpType
    hpool, lppool, h8pool, epool, opool, pairst = pools
    wt8, wt16, ct, ct8, bt, y16s, y8s = tiles
    c8 = l * CW8
    cb = l * CB
    p, q = t >> 1, t & 1          # pair index, parity (partition half)
    pb = 64 * q                   # partition base of this tile's [64,*] data

    def pair8(off, w):
        return wt8[0:128, c8 + off: c8 + off + 2 * w].rearrange(
            "p (k m) -> p k m", k=2)

    # ph0: mm0 both nets via DoubleRow (y8 | ones) x (A0 | b0row)
    tc.cur_priority = _PRIO(l, 0, t)
    merged = CFG["merge"]
    ps0 = {}
    for n in (0, 1):
        if merged:
            pp = hpool.tile([128, 2, BT], f32, tag="hp2", bufs=CFG["hp_bufs"],
                            name=f"ps0_{n}")
        for m in (0, 1):
            if not merged:
                pp = hpool.tile([128, BT], f32, tag="hp",
                                bufs=CFG["hp_bufs_u"], name=f"ps0_{n}{m}")
            off = c8 + n * 512 + m * 256
            lhsT = wt8[pb:pb + 64, off: off + 256].rearrange(
                "p (k m) -> p k m", k=2)
            dst = pp[:, m, :] if merged else pp[:, :]
            nc.tensor.matmul(dst, lhsT, y8s[p][pb:pb + 64, :, :],
                             start=True, stop=True, perf_mode=DR)
            ps0[(n, m)] = pp
        ps0[n] = pp

    # ph1: relu0 (b0 folded via the ones slot)
    tc.cur_priority = _PRIO(l, 1, t)
    h8 = {}
    for n in (0, 1):
        h8[n] = h8pool.tile([128, 2, BT], f8, tag="h8", name=f"h8_{n}")
        if merged:
            _relu(nc, CFG["relu0"][n],
                  h8[n][:, :, :].rearrange("p k n -> p (k n)"),
                  ps0[n][:, :, :].rearrange("p k n -> p (k n)"))
        else:
            for m in (0, 1):
                _relu(nc, _eng(0, n, m, t), h8[n][:, m, :],
                      ps0[(n, m)][:, :])

    # ph2: mm1 DoubleRow K=256 (+ b1 K=1 DR prebias when merged)
    tc.cur_priority = _PRIO(l, 2, t)
    ps1 = {}
    for n in (0, 1):
        if merged:
            pp = hpool.tile([128, 2, BT], f32, tag="hp2", bufs=CFG["hp_bufs"],
                            name=f"ps1_{n}")
        for m in (0, 1):
            if not merged:
                pp = hpool.tile([128, BT], f32, tag="hp",
                                bufs=CFG["hp_bufs_u"], name=f"ps1_{n}{m}")
            dst = pp[:, m, :] if merged else pp[:, :]
            if merged:
                boff = c8 + 2304 + (n * 2 + m) * 256
                nc.tensor.matmul(dst,
                                 wt8[0:1, boff: boff + 256].rearrange(
                                     "p (k m) -> p k m", k=2),
                                 ct8[0:1, :, :], start=True, stop=False,
                                 perf_mode=DR)
            lhsT = pair8(1024 + n * 512 + m * 256, 128)
            nc.tensor.matmul(dst, lhsT, h8[n][:, :, :],
                             start=(not merged), stop=True, perf_mode=DR)
            ps1[(n, m)] = pp
        ps1[n] = pp

    # ph3: relu1 (bias in psum when merged, engine bias otherwise)
    tc.cur_priority = _PRIO(l, 3, t)
    h18 = {}
    for n in (0, 1):
        h18[n] = h8pool.tile([128, 2, BT], f8, tag="h18", name=f"h18_{n}")
        if merged:
            _relu(nc, CFG["relu1"][n],
                  h18[n][:, :, :].rearrange("p k n -> p (k n)"),
                  ps1[n][:, :, :].rearrange("p k n -> p (k n)"))
        else:
            for m in (0, 1):
                bias_ap = bt[:, cb + n * 2 + m: cb + n * 2 + m + 1]
                _relu(nc, _eng(1, n, m, t), h18[n][:, m, :],
                      ps1[(n, m)][:, :], bias_ap)

    # ph4: mm2 into the pair's shared psum banks. Even tile -> rows 0:64
    # (DoubleRow), odd tile -> rows 64:128 (plain matmuls: DR can't write
    # partition base 64). loc psum accumulates -b2l (K=1) and +y16 (identity)
    # so it holds t = y - loc - b2l when done.
    tc.cur_priority = _PRIO(l, 4, t)
    if q == 0:
        psc = lppool.tile([128, BT], f32, tag="lsc")
        pt = lppool.tile([128, BT], f32, tag="lt")
        pairst[p] = (psc, pt)
        nc.tensor.matmul(psc[0:64], pair8(2048 + 128, 64), h18[1][:, :, :],
                         start=True, stop=True, perf_mode=DR)
        nc.tensor.matmul(pt[0:64], wt16[0:1, l * CW16: l * CW16 + 64],
                         ct[0:1, 64: 64 + BT], start=True, stop=False)
        nc.tensor.matmul(pt[0:64], pair8(2048, 64), h18[0][:, :, :],
                         start=False, stop=False, perf_mode=DR)
        nc.tensor.matmul(pt[0:64], ct[0:64, 0:64], y16s[p][0:64, :],
                         start=False, stop=True)
        return None
    psc, pt = pairst[p]
    # sc net (plain fp8 matmuls, k-chunks accumulated)
    soff = c8 + 2048 + 128
    for k in (0, 1):
        nc.tensor.matmul(psc[64:128],
                         wt8[0:128, soff + 64 * k: soff + 64 * (k + 1)],
                         h18[1][:, k, :], start=(k == 0), stop=(k == 1))
    # loc net: -b2l, -loc (2 plain chunks), +y16
    loff = c8 + 2048
    nc.tensor.matmul(pt[64:128], wt16[64:65, l * CW16: l * CW16 + 64],
                     ct[64:65, 64: 64 + BT], start=True, stop=False)
    for k in (0, 1):
        nc.tensor.matmul(pt[64:128],
                         wt8[0:128, loff + 64 * k: loff + 64 * (k + 1)],
                         h18[0][:, k, :], start=False, stop=False)
    nc.tensor.matmul(pt[64:128], ct[64:128, 0:64], y16s[p][64:128, :],
                     start=False, stop=True)

    # ph5: paired coupling over both tiles: e = exp(-sc - b2s); y' = t * e
    tc.cur_priority = _PRIO(l, 5, t)
    e16 = epool.tile([128, BT], f16, tag="e")
    nc.scalar.activation(e16[:], psc[:], AF.Exp,
                         bias=bt[:, cb + 4: cb + 5], scale=-1.0)
    if last:
        o32 = opool.tile([128, BT], f32, tag="o32", name="o32")
        nc.vector.tensor_tensor(o32[:], pt[:], e16[:], ALU.mult)
        return o32
    nc.vector.tensor_tensor(y16s[p][:, :], pt[:], e16[:], ALU.mult)
    # fp8 copy for the next layer's mm0 via casting DMA (off the engines)
    nc.gpsimd.dma_start(y8s[p][:, 0, :], y16s[p][:, :])
    return None


def _build():
    _patch_tile_drain(1)
    from contextlib import ExitStack

    f16, f32, f8 = DT.float16, DT.float32, DT.float8e4
    nc = bass.Bass(target_bir_lowering=False)
    u16_d = nc.declare_dram_parameter("u16", [64, BC], f16, isOutput=False)
    u8_d = nc.declare_dram_parameter("u8", [64, BC], f8, isOutput=False)
    w8_d = nc.declare_dram_parameter("w8", [L, 128, CW8], f8, isOutput=False)
    w16_d = nc.declare_dram_parameter("w16", [128, L * CW16], f16, isOutput=False)
    cn_d = nc.declare_dram_parameter("cn", [128, CONST_COLS], f16, isOutput=False)
    b_d = nc.declare_dram_parameter("bias", [128, L * CB], f32, isOutput=False)
    out_d = nc.declare_dram_parameter("out", [64, BC], f32, isOutput=True)

    with tile.TileContext(nc) as tc, ExitStack() as ctx:
        wpool = ctx.enter_context(tc.tile_pool(name="w", bufs=1))
        hpool = ctx.enter_context(tc.tile_pool(name="hp", bufs=1, space="PSUM"))
        lppool = ctx.enter_context(tc.tile_pool(name="lp", bufs=1, space="PSUM"))
        h8pool = ctx.enter_context(tc.tile_pool(name="h8", bufs=CFG["h8_bufs"]))
        epool = ctx.enter_context(tc.tile_pool(name="e", bufs=CFG["e_bufs"]))
        opool = ctx.enter_context(tc.tile_pool(name="o", bufs=3))

        wt8 = wpool.tile([128, L * CW8], f8)
        wt16 = wpool.tile([128, L * CW16], f16)
        ct = wpool.tile([128, CONST_COLS], f16)
        ct8 = wpool.tile([128, 2, BT], f8)      # fp8 ones (prebias moving)
        bt = wpool.tile([128, L * CB], f32)
        nc.gpsimd.memset(ct8[:, :, :], 1.0)

        # PE warmup: keep the clock-ramp monitor busy while DMAs land
        warm = wpool.tile([128, 128], f16)
        wps = lppool.tile([128, WARM_N], f32, tag="lsc", name="wps")
        nc.gpsimd.memset(warm[:], 0.0)
        for _ in range(CFG["warm"]):
            nc.tensor.matmul(wps[:, 0:WARM_N], warm[:, 0:WARM_N],
                             warm[:, 0:WARM_N], start=True, stop=True)

        nc.sync.dma_start(wt8[:, 0:CW8], w8_d[0])
        nc.sync.dma_start(ct[:], cn_d[:])
        nc.sync.dma_start(wt16[:], w16_d[:])
        nc.sync.dma_start(bt[:], b_d[:])

        y16s, y8s = [], []
        for p in range(NP):
            yt = wpool.tile([128, BT], f16, name=f"y16_{p}")
            nc.sync.dma_start(yt[0:64, :], u16_d[:, (2 * p) * BT:(2 * p + 1) * BT])
            nc.sync.dma_start(yt[64:128, :],
                              u16_d[:, (2 * p + 1) * BT:(2 * p + 2) * BT])
            y16s.append(yt)
        for p in range(NP):
            yt = wpool.tile([128, 2, BT], f8, name=f"y8_{p}")
            nc.sync.dma_start(yt[0:64, 0, :], u8_d[:, (2 * p) * BT:(2 * p + 1) * BT])
            nc.sync.dma_start(yt[64:128, 0, :],
                              u8_d[:, (2 * p + 1) * BT:(2 * p + 2) * BT])
            nc.gpsimd.memset(yt[:, 1, :], 1.0)
            y8s.append(yt)
        for l in range(1, L):
            nc.sync.dma_start(wt8[:, l * CW8:(l + 1) * CW8], w8_d[l])

        pairst = {}
        pools = (hpool, lppool, h8pool, epool, opool, pairst)
        tiles = (wt8, wt16, ct, ct8, bt, y16s, y8s)
        for l in range(L):
            for t in range(NT):
                o32 = _emit_layer(nc, tc, pools, tiles, l, t, l == L - 1)
                if o32 is not None:
                    p = t >> 1
                    nc.sync.dma_start(out_d[:, (2 * p) * BT:(2 * p + 1) * BT],
                                      o32[0:64, :])
                    nc.sync.dma_start(out_d[:, (2 * p + 1) * BT:(2 * p + 2) * BT],
                                      o32[64:128, :])
    _split_excess_waits(nc, maxw=1)
    return nc


_NC_CACHE = None


def _prep_blobs(inputs):
    M0, M1, M2 = _build_masks()
    w8 = np.zeros((L, 128, CW8), F8)
    w16 = np.zeros((128, L * CW16), np.float16)
    cn = np.zeros((128, CONST_COLS), np.float16)
    bb = np.zeros((128, L * CB), np.float32)
    cn[0:64, 0:64] = np.eye(64, dtype=np.float16)
    cn[64:128, 0:64] = np.eye(64, dtype=np.float16)
    cn[:, 64:CONST_COLS] = 1.0
    for l in range(L):
        for n, name in ((0, "loc"), (1, "scale")):
            A0 = (M0[l] * inputs[f"{name}_W0"][l]).astype(np.float32).T  # [64,256]
            A1 = (M1[l] * inputs[f"{name}_W1"][l]).astype(np.float32).T  # [256,256]
            A2 = (M2[l] * inputs[f"{name}_W2"][l]).astype(np.float32).T  # [256,64]
            b0 = inputs[f"{name}_b0"][l].astype(np.float32)
            b1 = inputs[f"{name}_b1"][l].astype(np.float32)
            b2 = inputs[f"{name}_b2"][l].astype(np.float32)
            if n == 0:
                A2 = -A2
                w16[0, l * CW16: l * CW16 + 64] = -b2.astype(np.float16)
                w16[64, l * CW16: l * CW16 + 64] = -b2.astype(np.float16)
            else:
                bb[0:64, l * CB + 4] = -b2
                bb[64:128, l * CB + 4] = -b2
            for m in (0, 1):
                off = n * 512 + m * 256
                a0c = A0[:, m * 128:(m + 1) * 128].astype(F8)
                w8[l, 0:64, off: off + 128] = a0c
                w8[l, 64:128, off: off + 128] = a0c
                b0c = b0[m * 128:(m + 1) * 128].astype(F8)
                w8[l, 0, off + 128: off + 256] = b0c
                w8[l, 64, off + 128: off + 256] = b0c
                off = 1024 + n * 512 + m * 256
                w8[l, :, off: off + 128] = \
                    A1[0:128, m * 128:(m + 1) * 128].astype(F8)
                w8[l, :, off + 128: off + 256] = \
                    A1[128:256, m * 128:(m + 1) * 128].astype(F8)
                bb[:, l * CB + n * 2 + m] = b1[m * 128:(m + 1) * 128]
                boff = 2304 + (n * 2 + m) * 256
                w8[l, 0, boff: boff + 128] = \
                    b1[m * 128:(m + 1) * 128].astype(F8)
            off = 2048 + n * 128
            w8[l, :, off: off + 64] = A2[0:128, :].astype(F8)
            w8[l, :, off + 64: off + 128] = A2[128:256, :].astype(F8)
    return w8, w16, cn, bb


def make_in_maps(inputs):
    inputs = {k: np.asarray(v) for k, v in inputs.items()}
    u = inputs["u"].astype(np.float32)            # [B, 64]
    w8, w16, cn, bb = _prep_blobs(inputs)
    uT16 = np.ascontiguousarray(u.T).astype(np.float16)
    uT8 = uT16.astype(F8)
    in_maps = []
    for c in range(NCORES):
        sl = slice(c * BC, (c + 1) * BC)
        in_maps.append({
            "u16": np.ascontiguousarray(uT16[:, sl]),
            "u8": np.ascontiguousarray(uT8[:, sl]),
            "w8": w8, "w16": w16, "cn": cn, "bias": bb,
        })
    return in_maps


def kernel(**inputs):
    global _NC_CACHE
    if _NC_CACHE is None:
        _NC_CACHE = _build()
    nc = _NC_CACHE
    in_maps = make_in_maps(inputs)
    res = run_bass_kernel_spmd(nc, in_maps, core_ids=list(range(NCORES)))
    out = np.empty((64, B), np.float32)
    for c in range(NCORES):
        out[:, c * BC:(c + 1) * BC] = res.results[c]["out"]
    return np.ascontiguousarray(out.T)


# revision 29
# speedup vs baseline: 1.4835x; 1.0006x over previous
import sys

for _p in ("/opt/trn_rl_repo",):
    if _p not in sys.path:
        sys.path.insert(0, _p)

import numpy as np
import ml_dtypes
import bass_rust
import concourse.bass as bass
import concourse.mybir as mybir
import concourse.tile as tile
from concourse.bass_utils import run_bass_kernel_spmd

DT = mybir.dt
F8 = ml_dtypes.float8_e4m3
DR = mybir.MatmulPerfMode.DoubleRow

# Problem constants (hardcoded from the nn_AutoFlow spec)
B, D, NH0, NH1, L = 32768, 64, 256, 256, 16
NCORES = 8
BC = B // NCORES          # 4096 samples per core
BT = 512                  # batch tile (free dim of activation tiles)
NT = BC // BT             # tiles per core
NP = NT // 2              # tile pairs: tile 2p -> partitions 0:64, 2p+1 -> 64:128
WARM_MMS = 40
WARM_N = 128

# fp8 weight blob column layout, per layer (DoubleRow pairs flattened as
# (k m) so rearrange("p (k m) -> p k m", k=2) recovers the pair)
# mm0 net n chunk m: [64p, 2x128] = (A0 m-chunk | b0 row) at n*512 + m*256
#   (rows 64:128 hold a copy for odd tiles whose y sits at partitions 64:128)
# mm1 net n chunk m: [128p, 2x128] = (A1 k0 mcols | A1 k1) at 1024+n*512+m*256
# mm2 net n:         [128p, 2x64]  = (A2 k0 | A2 k1)       at 2048 + n*128
# b1 prebias (n,m):   [1p, 2x128]  = (b1 chunk | zeros)     at 2304 + (n*2+m)*256
CW8 = 3328              # DRAM blob cols per layer
CW8S = 2304             # SBUF cols per layer when merge=False (b1 rows unused)

# fp16 blob: per layer col 0:64 = -b2l row (partitions 0 and 64)
CW16 = 64

# const fp16 blob [128, 576]: cols 0:64 = I64 (rows 0:64 and 64:128),
# cols 64:576 = 1.0
CONST_COLS = 576

# fp32 bias blob, per layer 5 cols:
# 0: b1_loc[0:128]  1: b1_loc[128:256]  2: b1_sc[0:128]  3: b1_sc[128:256]
# 4: -b2s (rows 0:64 and 64:128)
CB = 5

# engines for the per-(layer,tile) psum-draining ops. Only ACT ("act") and
# DVE ("dve") may touch PSUM; gpsimd/Pool is SBUF-only on this stack.
CFG = {
    "merge": False,           # merged 2-bank relus + b1 DR prebias
    "relu0": {0: "act", 1: "split"},
    "relu1": {0: "act", 1: "dve"},
    # unmerged per-(net,m) maps: interleave engines across m-chunks so both
    # engines drain a net's two psum banks in parallel
    "relu0u": {(0, 0): "act", (0, 1): "dve", (1, 0): "act", (1, 1): "dve"},
    "relu1u": {(0, 0): "dve", (0, 1): "act", (1, 0): "dve", (1, 1): "act"},
    "split_c": 135,           # "split": flattened cols [0:C] on ACT, rest DVE
    "prio": "pair",           # phase | tile | skew | pair
    "hp_bufs": 3,             # merged: [128,2,BT] tiles; unmerged: [128,BT]
    "hp_bufs_u": 6,
    "skew_off": (0, 3, 9, 12, 17, 18),
    # (phase, n, m, mod, rems, eng): for tiles with t % mod in rems, run
    # relu<phase> of chunk (n, m) on `eng` instead of the mapped engine
    "flips": [(1, 1, 0, 8, (4,), "act")],
    "h8_bufs": 10, "e_bufs": 8, "warm": 40,
}


def _patch_tile_drain(maxw=1):
    """walrus on this stack allows only 1 sync-wait on the kernel-tail Drain;
    split the TileContext drain's waits across a chain of drains."""
    from concourse.tile import ScopedClock

    def _drain_and_barrier(self, tick_clock, wait_clock):
        drain_inst = self.nc.sync.drain()
        wait_clock.add_sem_waits(
            drain_inst.ins, ScopedClock({None: tick_clock.global_clock})
        )
        inst = drain_inst.ins
        si = inst.sync_info
        if si is not None:
            waits = list(si.on_wait)
            ups = list(si.on_update)
            if len(waits) > maxw:
                chunks = [waits[i:i + maxw] for i in range(0, len(waits), maxw)]
                inst.sync_info = bass_rust.SyncInfo(on_wait=chunks[0], on_update=[])
                for j, chunk in enumerate(chunks[1:]):
                    extra = self.nc.sync.drain().ins
                    is_last = j == len(chunks) - 2
                    extra.sync_info = bass_rust.SyncInfo(
                        on_wait=chunk, on_update=ups if is_last else []
                    )
        self.nc.all_engine_barrier()
        assert self.sems is not None
        popped = self.nc._tile_sem_poison_stack.pop()
        assert popped is self._sem_poison
        self.nc.clear_and_free_semaphores(list(self.sems.allocated().values()))
        self.nc.all_engine_barrier()

    tile.TileContext._drain_and_barrier = _drain_and_barrier


_MAXW1_TYPES = ("InstDrain", "InstActivation")


def _split_excess_waits(nc, maxw=1):
    """walrus on this stack encodes very few semaphore-wait slots per
    instruction. Spill excess waits onto same-engine NoOps inserted just
    before the instruction (engine streams are in-order, so this is
    equivalent)."""
    for f in nc.m.functions:
        for bb in f.blocks:
            il = bb.instructions
            out = []
            for inst in il:
                si = getattr(inst, "sync_info", None)
                mw = 1 if type(inst).__name__ in _MAXW1_TYPES else maxw
                if si is not None and len(si.on_wait) > mw:
                    waits = list(si.on_wait)
                    ups = list(si.on_update)
                    chunks = [waits[i:i + mw] for i in range(0, len(waits), mw)]
                    for j, ch in enumerate(chunks[:-1]):
                        nop = mybir.InstNoOp(
                            name=f"{inst.name}-wsp{j}", ins=[], outs=[]
                        )
                        nop.engine = inst.engine
                        nop.sync_info = bass_rust.SyncInfo(on_wait=ch, on_update=[])
                        nc.register_instruction(nop, overwrite=True)
                        out.append(nop)
                    inst.sync_info = bass_rust.SyncInfo(
                        on_wait=chunks[-1], on_update=ups
                    )
                out.append(inst)
            if len(out) != len(il):
                il[:] = out


def _build_masks():
    mh0 = np.arange(NH0) % (D - 1)
    mh1 = np.arange(NH1) % (D - 1)
    M1 = (mh0[None, :] <= mh1[:, None]).astype(np.float32)
    M0s, M2s = [], []
    for l in range(L):
        perm = np.arange(D) if l % 2 == 0 else np.arange(D)[::-1]
        M0s.append((perm[None, :] <= mh0[:, None]).astype(np.float32))
        M2s.append((mh1[None, :] < perm[:, None]).astype(np.float32))
    return np.stack(M0s), np.broadcast_to(M1, (L,) + M1.shape).copy(), np.stack(M2s)


PRIO_GROUP = 8


def _PRIO(l, ph, t):
    mode = CFG["prio"]
    if mode == "phase":
        return ((l * 8 + ph) * NT + t) * 64
    if mode == "tile":
        return ((l * NT + t) * 8 + ph) * 64
    if mode == "skew":
        return ((l * NT + t) * 8 + CFG["skew_off"][ph]) * 64
    if mode == "pair":
        return ((((l * NP) + (t >> 1)) * 8 + ph) * 2 + (t & 1)) * 64
    if mode == "quad":
        return ((((l * (NT // 4)) + (t >> 2)) * 8 + ph) * 4 + (t & 3)) * 64
    if mode == "pairc":
        phx = {0: 0, 1: 1, 2: 2, 3: 3, 4: 4, 5: 4}[ph]
        return ((((l * NP) + (t >> 1)) * 8 + phx) * 2 + (t & 1)) * 64 + ph
    raise ValueError(mode)


def _eng(phase, n, m, t):
    for (ph, fn, fm, mod, rems, eng) in CFG["flips"]:
        if ph == phase and fn == n and fm == m and (t % mod) in rems:
            return eng
    return (CFG["relu0u"] if phase == 0 else CFG["relu1u"])[(n, m)]


def _relu(nc, eng, out_ap, in_ap, bias_ap=None):
    AF = mybir.ActivationFunctionType
    ALU = mybir.AluOpType
    if eng == "split":
        c = CFG["split_c"]
        _relu(nc, "act", out_ap[:, 0:c], in_ap[:, 0:c], bias_ap)
        _relu(nc, "dve", out_ap[:, c:], in_ap[:, c:], bias_ap)
        return
    if eng == "act":
        nc.scalar.activation(out_ap, in_ap, AF.Relu,
                             bias=0.0 if bias_ap is None else bias_ap)
    else:
        if bias_ap is None:
            nc.vector.tensor_scalar(out_ap, in_ap, 0.0, None, ALU.max)
        else:
            nc.vector.tensor_scalar(out_ap, in_ap, bias_ap, 0.0,
                                    ALU.add, ALU.max)


def _emit_layer(nc, tc, pools, tiles, l, t, last):
    f16, f32, f8 = DT.float16, DT.float32, DT.float8e4
    AF = mybir.ActivationFunctionType
    ALU = mybir.AluOpType
    hpool, lppool, h8pool, epool, opool, pairst = pools
    wt8, wt16, ct, ct8, bt, y16s, y8s = tiles
    c8 = l * CW8
    cb = l * CB
    p, q = t >> 1, t & 1          # pair index, parity (partition half)
    pb = 64 * q                   # partition base of this tile's [64,*] data

    def pair8(off, w):
        return wt8[0:128, c8 + off: c8 + off + 2 * w].rearrange(
            "p (k m) -> p k m", k=2)

    # ph0: mm0 both nets via DoubleRow (y8 | ones) x (A0 | b0row)
    tc.cur_priority = _PRIO(l, 0, t)
    merged = CFG["merge"]
    ps0 = {}
    for n in (0, 1):
        if merged:
            pp = hpool.tile([128, 2, BT], f32, tag="hp2", bufs=CFG["hp_bufs"],
                            name=f"ps0_{n}")
        for m in (0, 1):
            if not merged:
                pp = hpool.tile([128, BT], f32, tag="hp",
                                bufs=CFG["hp_bufs_u"], name=f"ps0_{n}{m}")
            off = c8 + n * 512 + m * 256
            lhsT = wt8[pb:pb + 64, off: off + 256].rearrange(
                "p (k m) -> p k m", k=2)
            dst = pp[:, m, :] if merged else pp[:, :]
            nc.tensor.matmul(dst, lhsT, y8s[p][pb:pb + 64, :, :],
                             start=True, stop=True, perf_mode=DR)
            ps0[(n, m)] = pp
        ps0[n] = pp

    # ph1: relu0 (b0 folded via the ones slot)
    tc.cur_priority = _PRIO(l, 1, t)
    h8 = {}
    for n in (0, 1):
        h8[n] = h8pool.tile([128, 2, BT], f8, tag="h8", name=f"h8_{n}")
        if merged:
            _relu(nc, CFG["relu0"][n],
                  h8[n][:, :, :].rearrange("p k n -> p (k n)"),
                  ps0[n][:, :, :].rearrange("p k n -> p (k n)"))
        else:
            for m in (0, 1):
                _relu(nc, _eng(0, n, m, t), h8[n][:, m, :],
                      ps0[(n, m)][:, :])

    # ph2: mm1 DoubleRow K=256 (+ b1 K=1 DR prebias when merged)
    tc.cur_priority = _PRIO(l, 2, t)
    ps1 = {}
    for n in (0, 1):
        if merged:
            pp = hpool.tile([128, 2, BT], f32, tag="hp2", bufs=CFG["hp_bufs"],
                            name=f"ps1_{n}")
        for m in (0, 1):
            if not merged:
                pp = hpool.tile([128, BT], f32, tag="hp",
                                bufs=CFG["hp_bufs_u"], name=f"ps1_{n}{m}")
            dst = pp[:, m, :] if merged else pp[:, :]
            if merged:
                boff = c8 + 2304 + (n * 2 + m) * 256
                nc.tensor.matmul(dst,
                                 wt8[0:1, boff: boff + 256].rearrange(
                                     "p (k m) -> p k m", k=2),
                                 ct8[0:1, :, :], start=True, stop=False,
                                 perf_mode=DR)
            lhsT = pair8(1024 + n * 512 + m * 256, 128)
            nc.tensor.matmul(dst, lhsT, h8[n][:, :, :],
                             start=(not merged), stop=True, perf_mode=DR)
            ps1[(n, m)] = pp
        ps1[n] = pp

    # ph3: relu1 (bias in psum when merged, engine bias otherwise)
    tc.cur_priority = _PRIO(l, 3, t)
    h18 = {}
    for n in (0, 1):
        h18[n] = h8pool.tile([128, 2, BT], f8, tag="h18", name=f"h18_{n}")
        if merged:
            _relu(nc, CFG["relu1"][n],
                  h18[n][:, :, :].rearrange("p k n -> p (k n)"),
                  ps1[n][:, :, :].rearrange("p k n -> p (k n)"))
        else:
            for m in (0, 1):
                bias_ap = bt[:, cb + n * 2 + m: cb + n * 2 + m + 1]
                _relu(nc, _eng(1, n, m, t), h18[n][:, m, :],
                      ps1[(n, m)][:, :], bias_ap)

    # ph4: mm2 into the pair's shared psum banks. Even tile -> rows 0:64
    # (DoubleRow), odd tile -> rows 64:128 (plain matmuls: DR can't write
    # partition base 64). loc psum accumulates -b2l (K=1) and +y16 (identity)
    # so it holds t = y - loc - b2l when done.
    tc.cur_priority = _PRIO(l, 4, t)
    if q == 0:
        psc = lppool.tile([128, BT], f32, tag="lsc")
        pt = lppool.tile([128, BT], f32, tag="lt")
        pairst[p] = (psc, pt)
        nc.tensor.matmul(psc[0:64], pair8(2048 + 128, 64), h18[1][:, :, :],
                         start=True, stop=True, perf_mode=DR)
        nc.tensor.matmul(pt[0:64], wt16[0:1, l * CW16: l * CW16 + 64],
                         ct[0:1, 64: 64 + BT], start=True, stop=False)
        nc.tensor.matmul(pt[0:64], pair8(2048, 64), h18[0][:, :, :],
                         start=False, stop=False, perf_mode=DR)
        nc.tensor.matmul(pt[0:64], ct[0:64, 0:64], y16s[p][0:64, :],
                         start=False, stop=True)
        return None
    psc, pt = pairst[p]
    # sc net (plain fp8 matmuls, k-chunks accumulated)
    soff = c8 + 2048 + 128
    for k in (0, 1):
        nc.tensor.matmul(psc[64:128],
                         wt8[0:128, soff + 64 * k: soff + 64 * (k + 1)],
                         h18[1][:, k, :], start=(k == 0), stop=(k == 1))
    # loc net: -b2l, -loc (2 plain chunks), +y16
    loff = c8 + 2048
    nc.tensor.matmul(pt[64:128], wt16[64:65, l * CW16: l * CW16 + 64],
                     ct[64:65, 64: 64 + BT], start=True, stop=False)
    for k in (0, 1):
        nc.tensor.matmul(pt[64:128],
                         wt8[0:128, loff + 64 * k: loff + 64 * (k + 1)],
                         h18[0][:, k, :], start=False, stop=False)
    nc.tensor.matmul(pt[64:128], ct[64:128, 0:64], y16s[p][64:128, :],
                     start=False, stop=True)

    # ph5: paired coupling over both tiles: e = exp(-sc - b2s); y' = t * e
    tc.cur_priority = _PRIO(l, 5, t)
    e16 = epool.tile([128, BT], f16, tag="e")
    nc.scalar.activation(e16[:], psc[:], AF.Exp,
                         bias=bt[:, cb + 4: cb + 5], scale=-1.0)
    if last:
        o32 = opool.tile([128, BT], f32, tag="o32", name="o32")
        nc.vector.tensor_tensor(o32[:], pt[:], e16[:], ALU.mult)
        return o32
    nc.vector.tensor_tensor(y16s[p][:, :], pt[:], e16[:], ALU.mult)
    # fp8 copy for the next layer's mm0 via casting DMA (off the engines)
    nc.gpsimd.dma_start(y8s[p][:, 0, :], y16s[p][:, :])
    return None


def _build():
    _patch_tile_drain(1)
    from contextlib import ExitStack

    f16, f32, f8 = DT.float16, DT.float32, DT.float8e4
    nc = bass.Bass(target_bir_lowering=False)
    u16_d = nc.declare_dram_parameter("u16", [64, BC], f16, isOutput=False)
    u8_d = nc.declare_dram_parameter("u8", [64, BC], f8, isOutput=False)
    w8_d = nc.declare_dram_parameter("w8", [L, 128, CW8], f8, isOutput=False)
    w16_d = nc.declare_dram_parameter("w16", [128, L * CW16], f16, isOutput=False)
    cn_d = nc.declare_dram_parameter("cn", [128, CONST_COLS], f16, isOutput=False)
    b_d = nc.declare_dram_parameter("bias", [128, L * CB], f32, isOutput=False)
    out_d = nc.declare_dram_parameter("out", [64, BC], f32, isOutput=True)

    with tile.TileContext(nc) as tc, ExitStack() as ctx:
        wpool = ctx.enter_context(tc.tile_pool(name="w", bufs=1))
        hpool = ctx.enter_context(tc.tile_pool(name="hp", bufs=1, space="PSUM"))
        lppool = ctx.enter_context(tc.tile_pool(name="lp", bufs=1, space="PSUM"))
        h8pool = ctx.enter_context(tc.tile_pool(name="h8", bufs=CFG["h8_bufs"]))
        epool = ctx.enter_context(tc.tile_pool(name="e", bufs=CFG["e_bufs"]))
        opool = ctx.enter_context(tc.tile_pool(name="o", bufs=3))

        wt8 = wpool.tile([128, L * CW8], f8)
        wt16 = wpool.tile([128, L * CW16], f16)
        ct = wpool.tile([128, CONST_COLS], f16)
        ct8 = wpool.tile([128, 2, BT], f8)      # fp8 ones (prebias moving)
        bt = wpool.tile([128, L * CB], f32)
        nc.gpsimd.memset(ct8[:, :, :], 1.0)

        # PE warmup: keep the clock-ramp monitor busy while DMAs land
        warm = wpool.tile([128, 128], f16)
        wps = lppool.tile([128, WARM_N], f32, tag="lsc", name="wps")
        nc.gpsimd.memset(warm[:], 0.0)
        for _ in range(CFG["warm"]):
            nc.tensor.matmul(wps[:, 0:WARM_N], warm[:, 0:WARM_N],
                             warm[:, 0:WARM_N], start=True, stop=True)

        nc.sync.dma_start(wt8[:, 0:CW8], w8_d[0])
        nc.sync.dma_start(ct[:], cn_d[:])
        nc.sync.dma_start(wt16[:], w16_d[:])
        nc.sync.dma_start(bt[:], b_d[:])

        y16s, y8s = [], []
        for p in range(NP):
            yt = wpool.tile([128, BT], f16, name=f"y16_{p}")
            nc.sync.dma_start(yt[0:64, :], u16_d[:, (2 * p) * BT:(2 * p + 1) * BT])
            nc.sync.dma_start(yt[64:128, :],
                              u16_d[:, (2 * p + 1) * BT:(2 * p + 2) * BT])
            y16s.append(yt)
        for p in range(NP):
            yt = wpool.tile([128, 2, BT], f8, name=f"y8_{p}")
            nc.sync.dma_start(yt[0:64, 0, :], u8_d[:, (2 * p) * BT:(2 * p + 1) * BT])
            nc.sync.dma_start(yt[64:128, 0, :],
                              u8_d[:, (2 * p + 1) * BT:(2 * p + 2) * BT])
            nc.gpsimd.memset(yt[:, 1, :], 1.0)
            y8s.append(yt)
        for l in range(1, L):
            nc.sync.dma_start(wt8[:, l * CW8:(l + 1) * CW8], w8_d[l])

        pairst = {}
        pools = (hpool, lppool, h8pool, epool, opool, pairst)
        tiles = (wt8, wt16, ct, ct8, bt, y16s, y8s)
        for l in range(L):
            for t in range(NT):
                o32 = _emit_layer(nc, tc, pools, tiles, l, t, l == L - 1)
                if o32 is not None:
                    p = t >> 1
                    nc.sync.dma_start(out_d[:, (2 * p) * BT:(2 * p + 1) * BT],
                                      o32[0:64, :])
                    nc.sync.dma_start(out_d[:, (2 * p + 1) * BT:(2 * p + 2) * BT],
                                      o32[64:128, :])
    _split_excess_waits(nc, maxw=1)
    return nc


_NC_CACHE = None


def _prep_blobs(inputs):
    M0, M1, M2 = _build_masks()
    w8 = np.zeros((L, 128, CW8), F8)
    w16 = np.zeros((128, L * CW16), np.float16)
    cn = np.zeros((128, CONST_COLS), np.float16)
    bb = np.zeros((128, L * CB), np.float32)
    cn[0:64, 0:64] = np.eye(64, dtype=np.float16)
    cn[64:128, 0:64] = np.eye(64, dtype=np.float16)
    cn[:, 64:CONST_COLS] = 1.0
    for l in range(L):
        for n, name in ((0, "loc"), (1, "scale")):
            A0 = (M0[l] * inputs[f"{name}_W0"][l]).astype(np.float32).T  # [64,256]
            A1 = (M1[l] * inputs[f"{name}_W1"][l]).astype(np.float32).T  # [256,256]
            A2 = (M2[l] * inputs[f"{name}_W2"][l]).astype(np.float32).T  # [256,64]
            b0 = inputs[f"{name}_b0"][l].astype(np.float32)
            b1 = inputs[f"{name}_b1"][l].astype(np.float32)
            b2 = inputs[f"{name}_b2"][l].astype(np.float32)
            if n == 0:
                A2 = -A2
                w16[0, l * CW16: l * CW16 + 64] = -b2.astype(np.float16)
                w16[64, l * CW16: l * CW16 + 64] = -b2.astype(np.float16)
            else:
                bb[0:64, l * CB + 4] = -b2
                bb[64:128, l * CB + 4] = -b2
            for m in (0, 1):
                off = n * 512 + m * 256
                a0c = A0[:, m * 128:(m + 1) * 128].astype(F8)
                w8[l, 0:64, off: off + 128] = a0c
                w8[l, 64:128, off: off + 128] = a0c
                b0c = b0[m * 128:(m + 1) * 128].astype(F8)
                w8[l, 0, off + 128: off + 256] = b0c
                w8[l, 64, off + 128: off + 256] = b0c
                off = 1024 + n * 512 + m * 256
                w8[l, :, off: off + 128] = \
                    A1[0:128, m * 128:(m + 1) * 128].astype(F8)
                w8[l, :, off + 128: off + 256] = \
                    A1[128:256, m * 128:(m + 1) * 128].astype(F8)
                bb[:, l * CB + n * 2 + m] = b1[m * 128:(m + 1) * 128]
                boff = 2304 + (n * 2 + m) * 256
                w8[l, 0, boff: boff + 128] = \
                    b1[m * 128:(m + 1) * 128].astype(F8)
            off = 2048 + n * 128
            w8[l, :, off: off + 64] = A2[0:128, :].astype(F8)
            w8[l, :, off + 64: off + 128] = A2[128:256, :].astype(F8)
    return w8, w16, cn, bb


def make_in_maps(inputs):
    inputs = {k: np.asarray(v) for k, v in inputs.items()}
    u = inputs["u"].astype(np.float32)            # [B, 64]
    w8, w16, cn, bb = _prep_blobs(inputs)
    uT16 = np.ascontiguousarray(u.T).astype(np.float16)
    uT8 = uT16.astype(F8)
    in_maps = []
    for c in range(NCORES):
        sl = slice(c * BC, (c + 1) * BC)
        in_maps.append({
            "u16": np.ascontiguousarray(uT16[:, sl]),
            "u8": np.ascontiguousarray(uT8[:, sl]),
            "w8": w8, "w16": w16, "cn": cn, "bias": bb,
        })
    return in_maps


def kernel(**inputs):
    global _NC_CACHE
    if _NC_CACHE is None:
        _NC_CACHE = _build()
    nc = _NC_CACHE
    in_maps = make_in_maps(inputs)
    res = run_bass_kernel_spmd(nc, in_maps, core_ids=list(range(NCORES)))
    out = np.empty((64, B), np.float32)
    for c in range(NCORES):
        out[:, c * BC:(c + 1) * BC] = res.results[c]["out"]
    return np.ascontiguousarray(out.T)
